# revision 1
# baseline (speedup 1.0000x reference)
"""Multi-head attention (B=4, S=2048, E=1024, H=16) on 8 trn2 NeuronCores.

Sharding: data-parallel over B (4) x tensor-parallel over H (2 halves of 8
heads). Core c handles batch c//2, head-half c%2. Column-parallel qkv_proj,
row-parallel out_proj; the all-reduce of the two partial outputs per batch is
done on the host during unshard (a sum of two arrays), as is the final
transpose (the device emits out^T to keep DMA writes contiguous).

Device kernel (identical program on all 8 cores, fp32r matmuls):
  phase 1/2: qk^T = Wqk_loc @ x^T  [1024, 2048] and v = x @ Wv_loc^T + bv
             [2048, 512] (bias via a K=1 ones-row matmul)
  phase 3:   per head pair, flash-style over 128-key tiles:
             scores^T pairs row-packed at partitions 0/64, ACT exp with the
             1/sqrt(E) scale folded in, PV matmul with stationary [v | 1]
             (even head, M=65: ctx at partitions 0-63, sums at 64) or
             [1 | 0*63 | v] (odd head, M=128: sums at 0, ctx at 64-127) so
             the softmax denominator rides along for free; normalization by
             reciprocal of a DRAM-bounce partition-broadcast of the sums row
  phase 4:   out^T partial = Wout_loc^T-stationary matmuls + bias (bout on
             even cores only, zeros on odd, so the host sum adds it once)
"""
import sys

import numpy as np

sys.path.insert(0, "/opt/trn_rl_repo")

import concourse.bacc as bacc
import concourse.mybir as mybir
import concourse.tile as tile
from concourse.bass_utils import run_bass_kernel_spmd
from concourse.tile_rust import add_dep_helper

F32 = mybir.dt.float32
F32R = mybir.dt.float32r
EXP = mybir.ActivationFunctionType.Exp

B, S, E, H, HD = 4, 2048, 1024, 16, 64
HL = 8            # heads per core
SCALE = 1.0 / np.sqrt(E).astype(np.float32)

# cons layout: [0:128] ones, [128:640] bv, [640:1412] v1 pad pattern
ONES_OFF, BV_OFF, VPAD_OFF, CONS_LEN = 0, 128, 640, 1412
V1W = 4 * 65 + 4 * 128   # 772 cols per key-tile block in v1


def build_nc():
    nc = bacc.Bacc("TRN2", target_bir_lowering=False, debug=False, num_devices=8)
    xw_d = nc.declare_dram_parameter("xw", [E, 3584], F32, isOutput=False)
    bqk_d = nc.declare_dram_parameter("bqk", [E, 1], F32, isOutput=False)
    cons_d = nc.declare_dram_parameter("cons", [1, CONS_LEN], F32, isOutput=False)
    wo_d = nc.declare_dram_parameter("wo", [512, E], F32, isOutput=False)
    bout_d = nc.declare_dram_parameter("bout", [E, 1], F32, isOutput=False)
    out_d = nc.declare_dram_parameter("outT", [E, S], F32, isOutput=True)
    rb = nc.dram_tensor("rb", [8, S], F32)   # sums bounce rows

    with tile.TileContext(nc) as tc:
      with tc.tile_pool(name="pp", bufs=1) as pp:
        bqk_s = pp.tile([128, 8, 1], F32)
        bout_s = pp.tile([128, 8, 1], F32)
        cons_s = pp.tile([1, CONS_LEN], F32R)
        nc.gpsimd.dma_start(out=bqk_s, in_=bqk_d[:, :].rearrange("(m p) o -> p m o", p=128))
        nc.gpsimd.dma_start(out=bout_s, in_=bout_d[:, :].rearrange("(m p) o -> p m o", p=128))
        nc.gpsimd.dma_start(out=cons_s, in_=cons_d[:, :].bitcast(F32R))
        # warm the ACT exp table set during phase 1/2 (load is ~2.7us and would
        # otherwise land at the first attention exp, on the critical path)
        warm = pp.tile([1, 1], F32)
        nc.scalar.activation(out=warm, in_=bqk_s[0:1, 0, 0:1], func=EXP)

        with tc.tile_pool(name="pa", bufs=1) as pa:
            qk_s = pa.tile([128, 8, S], F32R)     # qk^T: m-tile 0-3 q, 4-7 k
            v1_s = pa.tile([128, 16, V1W], F32R)  # per key-tile [v|1]x4, [1|0|v]x4

            # ---- phases 1+2: qk^T and v
            with tc.tile_pool(name="p12", bufs=1) as p12, \
                 tc.tile_pool(name="ps12", bufs=1, space="PSUM") as ps12:
                wv_s = p12.tile([128, 8, 512], F32R)
                for kt in range(8):
                    nc.gpsimd.dma_start(
                        out=wv_s[:, kt, :],
                        in_=xw_d[kt * 128:(kt + 1) * 128, 3072:3584].bitcast(F32R))
                # v1 pad pattern (ones + zero pads; v cols overwritten by evicts)
                # -- after wv on the gpsimd queue: wv is needed ~15us earlier
                for jt in range(16):
                    nc.gpsimd.dma_start(
                        out=v1_s[:, jt, :],
                        in_=cons_d[0:1, VPAD_OFF:VPAD_OFF + V1W].bitcast(F32R)
                        .to_broadcast([128, V1W]))
                for mh in range(2):
                    wqk_s = p12.tile([128, 8, 512], F32R, tag="wqk", bufs=2)
                    for ic in range(4):
                        xc_s = p12.tile([128, 8, 512], F32R, tag="xc", bufs=2)
                        for kt in range(8):
                            if ic == 0:
                                nc.sync.dma_start(
                                    out=wqk_s[:, kt, :],
                                    in_=xw_d[kt * 128:(kt + 1) * 128,
                                             mh * 512:(mh + 1) * 512].bitcast(F32R))
                            nc.sync.dma_start(
                                out=xc_s[:, kt, :],
                                in_=xw_d[kt * 128:(kt + 1) * 128,
                                         1024 + ic * 512:1024 + (ic + 1) * 512]
                                .bitcast(F32R))
                        for m in range(4):
                            pq = ps12.tile([128, 512], F32, tag="pq", bufs=4)
                            for kt in range(8):
                                nc.tensor.matmul(
                                    out=pq, lhsT=wqk_s[:, kt, m * 128:(m + 1) * 128],
                                    rhs=xc_s[:, kt, :],
                                    start=(kt == 0), stop=(kt == 7))
                            nc.vector.tensor_scalar_add(
                                qk_s[:, mh * 4 + m, ic * 512:(ic + 1) * 512],
                                pq, bqk_s[:, mh * 4 + m, 0:1])
                        if mh == 0:
                            for st in range(4):
                                jt = ic * 4 + st
                                pv = ps12.tile([128, 512], F32, tag="pv", bufs=3)
                                for kt in range(8):
                                    nc.tensor.matmul(
                                        out=pv,
                                        lhsT=xc_s[:, kt, st * 128:(st + 1) * 128],
                                        rhs=wv_s[:, kt, :],
                                        start=(kt == 0), stop=False)
                                nc.tensor.matmul(
                                    out=pv, lhsT=cons_s[0:1, ONES_OFF:ONES_OFF + 128],
                                    rhs=cons_s[0:1, BV_OFF:BV_OFF + 512],
                                    start=False, stop=True)
                                # evict: even heads -> [v|1] blocks, odd -> [1|0|v]
                                nc.vector.tensor_copy(
                                    v1_s[:, jt, 0:260]
                                    .rearrange("p (b c) -> p b c", c=65)[:, :, 0:64],
                                    pv[:, :].rearrange("p (b t d) -> p b t d", t=2, d=64)
                                    [:, :, 0, :])
                                nc.vector.tensor_copy(
                                    v1_s[:, jt, 260:V1W]
                                    .rearrange("p (b c) -> p b c", c=128)[:, :, 64:128],
                                    pv[:, :].rearrange("p (b t d) -> p b t d", t=2, d=64)
                                    [:, :, 1, :])

            # ---- phase 3: attention per head pair
            with tc.tile_pool(name="pc", bufs=1) as pc:
                ctx_t = [pc.tile([128, S], F32R, name=f"ctx{i}", tag=f"ctx{i}",
                                 bufs=1) for i in range(4)]
                wo_s = pc.tile([128, 4, E], F32R)
                for ct in range(4):
                    nc.sync.dma_start(
                        out=wo_s[:, ct, :],
                        in_=wo_d[ct * 128:(ct + 1) * 128, :].bitcast(F32R))
                with tc.tile_pool(name="ps3", bufs=1, space="PSUM") as ps3:
                  with tc.tile_pool(name="p3", bufs=1) as p3:
                    for p in range(4):
                        for icp in range(2):
                            s_e = ps3.tile([128, 1024], F32, tag="s_e", bufs=1)
                            s_o = ps3.tile([128, 1024], F32, tag="s_o", bufs=1)
                            pv_e = ps3.tile([65, 1024], F32, tag="pv_e", bufs=1)
                            pv_o = ps3.tile([128, 1024], F32, tag="pv_o", bufs=1)
                            for jt in range(16):
                                for ih in range(2):
                                    icol = icp * 1024 + ih * 512
                                    nc.tensor.matmul(
                                        out=s_e[:, ih * 512:(ih + 1) * 512],
                                        lhsT=qk_s[0:64, 4 + p, jt * 128:(jt + 1) * 128],
                                        rhs=qk_s[0:64, p, icol:icol + 512],
                                        start=True, stop=True)
                                    nc.tensor.matmul(
                                        out=s_o[:, ih * 512:(ih + 1) * 512],
                                        lhsT=qk_s[64:128, 4 + p, jt * 128:(jt + 1) * 128],
                                        rhs=qk_s[64:128, p, icol:icol + 512],
                                        start=True, stop=True)
                                e_e = p3.tile([128, 1024], F32R, tag="e", bufs=2)
                                nc.scalar.activation(out=e_e, in_=s_e, func=EXP,
                                                     scale=float(SCALE))
                                e_o = p3.tile([128, 1024], F32R, tag="e", bufs=2)
                                nc.scalar.activation(out=e_o, in_=s_o, func=EXP,
                                                     scale=float(SCALE))
                                for ih in range(2):
                                    sl = slice(ih * 512, (ih + 1) * 512)
                                    nc.tensor.matmul(
                                        out=pv_e[:, sl],
                                        lhsT=v1_s[:, jt, p * 65:p * 65 + 65],
                                        rhs=e_e[:, sl],
                                        start=(jt == 0), stop=(jt == 15))
                                    nc.tensor.matmul(
                                        out=pv_o[:, sl],
                                        lhsT=v1_s[:, jt, 260 + p * 128:260 + (p + 1) * 128],
                                        rhs=e_o[:, sl],
                                        start=(jt == 0), stop=(jt == 15))
                            # evict pv to sbuf (frees psum), then normalize
                            pvt_e = p3.tile([65, 1024], F32, tag="pvt_e", bufs=1)
                            pvt_o = p3.tile([128, 1024], F32, tag="pvt_o", bufs=1)
                            nc.vector.tensor_copy(pvt_e, pv_e)
                            nc.vector.tensor_copy(pvt_o, pv_o)
                            ic_sl = slice(icp * 1024, (icp + 1) * 1024)
                            st_e = nc.sync.dma_start(out=rb[2 * p:2 * p + 1, ic_sl],
                                                     in_=pvt_e[64:65, :])
                            st_o = nc.sync.dma_start(out=rb[2 * p + 1:2 * p + 2, ic_sl],
                                                     in_=pvt_o[0:1, :])
                            rep = p3.tile([128, 1024], F32, tag="rep", bufs=1)
                            ld_e = nc.gpsimd.dma_start(
                                out=rep[0:64, :],
                                in_=rb[2 * p:2 * p + 1, ic_sl].to_broadcast([64, 1024]))
                            ld_o = nc.gpsimd.dma_start(
                                out=rep[64:128, :],
                                in_=rb[2 * p + 1:2 * p + 2, ic_sl].to_broadcast([64, 1024]))
                            add_dep_helper(ld_e.ins, st_e.ins, sync=True, reason="raw_e")
                            add_dep_helper(ld_o.ins, st_o.ins, sync=True, reason="raw_o")
                            rrec = p3.tile([128, 1024], F32, tag="rrec", bufs=1)
                            rscr = p3.tile([128, 1024], F32, tag="rscr", bufs=1)
                            nc.vector.reciprocal_approx_accurate(
                                out=rrec, in_=rep, scratch=rscr)
                            nc.vector.tensor_mul(ctx_t[p][0:64, ic_sl],
                                                 pvt_e[0:64, :], rrec[0:64, :])
                            nc.vector.tensor_mul(ctx_t[p][64:128, ic_sl],
                                                 pvt_o[64:128, :], rrec[64:128, :])

                  # ---- phase 4: out projection (partial), written as out^T
                  # (still inside ps3: po reuses the s_e/s_o tag slots so there
                  # is no psum pool transition barrier)
                  with tc.tile_pool(name="p4", bufs=1) as p4:
                    for et in range(8):
                        for i4 in range(4):
                            k = et * 4 + i4
                            po = ps3.tile([128, 512], F32,
                                          tag=("s_e" if k % 2 == 0 else "s_o"),
                                          bufs=1, name=f"po_{k}")
                            for ct in range(4):
                                nc.tensor.matmul(
                                    out=po, lhsT=wo_s[:, ct, et * 128:(et + 1) * 128],
                                    rhs=ctx_t[ct][:, i4 * 512:(i4 + 1) * 512],
                                    start=(ct == 0), stop=(ct == 3))
                            ot = p4.tile([128, 512], F32, tag="ot", bufs=4)
                            nc.vector.tensor_scalar_add(ot, po, bout_s[:, et, 0:1])
                            nc.sync.dma_start(
                                out=out_d[et * 128:(et + 1) * 128,
                                          i4 * 512:(i4 + 1) * 512],
                                in_=ot)
    nc.compile()
    return nc


_NC = None


def _get_nc():
    global _NC
    if _NC is None:
        _NC = build_nc()
    return _NC


def make_in_maps(query, Wqkv, bqkv, Wout, bout):
    query = np.asarray(query, dtype=np.float32)
    Wqkv = np.asarray(Wqkv, dtype=np.float32)
    bqkv = np.asarray(bqkv, dtype=np.float32)
    Wout = np.asarray(Wout, dtype=np.float32)
    bout = np.asarray(bout, dtype=np.float32)

    in_maps = []
    for c in range(8):
        b, hh = c // 2, c % 2
        heads = np.arange(hh * HL, hh * HL + HL)
        dims = (heads[:, None] * HD + np.arange(HD)[None, :]).reshape(-1)  # [512]
        q_rows, k_rows, v_rows = dims, E + dims, 2 * E + dims

        xw = np.empty((E, 3584), np.float32)
        xw[:, 0:512] = Wqkv[q_rows].T
        xw[:, 512:1024] = Wqkv[k_rows].T
        xw[:, 1024:3072] = query[b].T
        xw[:, 3072:3584] = Wqkv[v_rows].T

        bqk = np.concatenate([bqkv[q_rows], bqkv[k_rows]]).reshape(E, 1)

        cons = np.zeros((1, CONS_LEN), np.float32)
        cons[0, ONES_OFF:ONES_OFF + 128] = 1.0
        cons[0, BV_OFF:BV_OFF + 512] = bqkv[v_rows]
        vpad = np.zeros(V1W, np.float32)
        for i in range(4):
            vpad[i * 65 + 64] = 1.0          # even-head ones col
            vpad[260 + i * 128] = 1.0        # odd-head ones col
        cons[0, VPAD_OFF:VPAD_OFF + V1W] = vpad

        wo = np.ascontiguousarray(Wout[:, dims].T)          # [512, E]
        bo = (bout if hh == 0 else np.zeros_like(bout)).reshape(E, 1)

        in_maps.append({
            "xw": xw, "bqk": np.ascontiguousarray(bqk),
            "cons": cons, "wo": wo, "bout": np.ascontiguousarray(bo),
        })
    return in_maps


def gather(results):
    out = np.empty((B, S, E), np.float32)
    for b in range(B):
        acc = results[2 * b]["outT"] + results[2 * b + 1]["outT"]   # [E, S]
        out[b] = acc.T
    return out


def kernel(query, key, value, Wqkv, bqkv, Wout, bout):
    # key/value are unused by the reference module (qkv all from query)
    nc = _get_nc()
    in_maps = make_in_maps(query, Wqkv, bqkv, Wout, bout)
    res = run_bass_kernel_spmd(nc, in_maps, list(range(8)))
    return gather(res.results)



# revision 5
# speedup vs baseline: 1.1672x; 1.1672x over previous
"""Multi-head attention (B=4, S=2048, E=1024, H=16) on 8 trn2 NeuronCores.

Sharding: data-parallel over B (4) x tensor-parallel over H (2 halves of 8
heads). Core c handles batch c//2, head-half c%2. Column-parallel qkv_proj,
row-parallel out_proj; the all-reduce of the two partial outputs per batch is
done on the host during unshard (a sum of two arrays), as is the final
transpose (the device emits out^T to keep DMA writes contiguous).

Device kernel v2 (bf16 matmuls, fp32 psum): per head-pair p, JIT qk-proj
(bf16, out evicted bf16); per head: scores^T per key-tile in [128 keys, 1024
queries] psum tiles, ACT exp -> e bf16 (scale 1/sqrt(E) folded); PV runs
TRANSPOSED: stationary = e-tile [128 keys, 128 queries], moving = [v_h | 1]
bf16 [128, 65], so psum accumulates [128 q, 64 ctx | softmax-denominator].
Normalization is then a per-partition reciprocal + tensor_scalar_mul (no
DRAM-bounce broadcast). ctx [q, d] tiles are transposed to [d, q] with the
DMA xbar (dma_start_transpose, off the PE critical path), then row-parallel
out-proj emits out^T partials. v-proj overlaps head-0's exps; out-proj's
first query-half overlaps the last head's second-half exps.
"""
import sys

import numpy as np

sys.path.insert(0, "/opt/trn_rl_repo")

import ml_dtypes

import concourse.bacc as bacc
import concourse.mybir as mybir
import concourse.tile as tile
from concourse.bass_utils import run_bass_kernel_spmd

F32 = mybir.dt.float32
BF16 = mybir.dt.bfloat16
EXP = mybir.ActivationFunctionType.Exp

B, S, E, H, HD = 4, 2048, 1024, 16, 64
HL = 8            # heads per core
SCALE = float(1.0 / np.sqrt(np.float32(E)))

# cons layout (bf16 row): [0:128] ones, [128:640] bv, [640:1160] v1 pad row
ONES_OFF, BV_OFF, VPAD_OFF, CONS_LEN = 0, 128, 640, 1160


def build_nc():
    nc = bacc.Bacc("TRN2", target_bir_lowering=False, debug=False, num_devices=8)
    # xw cols: [0:1024] per-pair [Wq_p^T | Wk_p^T] (4 x 256), [1024:3072] x^T,
    # [3072:3584] Wv_loc^T -- all bf16
    xw_d = nc.declare_dram_parameter("xw", [E, 3584], BF16, isOutput=False)
    bqk_d = nc.declare_dram_parameter("bqk", [E, 1], F32, isOutput=False)
    cons_d = nc.declare_dram_parameter("cons", [1, CONS_LEN], BF16, isOutput=False)
    wo_d = nc.declare_dram_parameter("wo", [512, E], BF16, isOutput=False)
    bout_d = nc.declare_dram_parameter("bout", [E, 1], F32, isOutput=False)
    out_d = nc.declare_dram_parameter("outT", [E, S], F32, isOutput=True)

    with tile.TileContext(nc) as tc:
      with tc.tile_pool(name="pp", bufs=1) as pp:
        bqk_s = pp.tile([128, 8, 1], F32)
        bout_s = pp.tile([128, 8, 1], F32)
        cons_s = pp.tile([1, CONS_LEN], BF16)
        nc.gpsimd.dma_start(out=bqk_s, in_=bqk_d[:, :].rearrange("(m p) o -> p m o", p=128))
        nc.gpsimd.dma_start(out=bout_s, in_=bout_d[:, :].rearrange("(m p) o -> p m o", p=128))
        nc.gpsimd.dma_start(out=cons_s, in_=cons_d[:, :])
        # warm the ACT exp table (load is ~1.3us; keep it off the critical path)
        warm = pp.tile([1, 1], F32)
        nc.scalar.activation(out=warm, in_=bqk_s[0:1, 0, 0:1], func=EXP)

        with tc.tile_pool(name="pa", bufs=1) as pa:
            # persistent phase-3 state
            qk_s = pa.tile([128, 2, 2, S], BF16)        # [slot, q|k, tok]
            v1_s = pa.tile([128, 16, 520], BF16)        # per jt: 8 x [v_h(64) | 1]
            e_s = [pa.tile([128, 16, 1024], BF16, name=f"e{i}") for i in range(3)]
            ctq_s = pa.tile([128, 2, 16, 128], BF16)    # [slot, qb, d-pair]
            ctx_s = pa.tile([128, 4, S], BF16)          # ctx^T per pair [d, q]

            with tc.tile_pool(name="ps", bufs=1, space="PSUM") as ps:
              with tc.tile_pool(name="px", bufs=1) as px:
                x_s = px.tile([128, 8, S], BF16)
                wv_s = px.tile([128, 8, 512], BF16)
                wqk0_t = px.tile([128, 8, 256], BF16, tag="wqk", bufs=2)
                for kt in range(8):
                    nc.sync.dma_start(
                        out=wqk0_t[:, kt, :],
                        in_=xw_d[kt * 128:(kt + 1) * 128, 0:256])
                for ic in range(4):
                    for kt in range(8):
                        nc.sync.dma_start(
                            out=x_s[:, kt, ic * 512:(ic + 1) * 512],
                            in_=xw_d[kt * 128:(kt + 1) * 128,
                                     1024 + ic * 512:1024 + (ic + 1) * 512])
                for kt in range(8):
                    nc.gpsimd.dma_start(
                        out=wv_s[:, kt, :],
                        in_=xw_d[kt * 128:(kt + 1) * 128, 3072:3584])
                for jt in range(16):
                    nc.gpsimd.dma_start(
                        out=v1_s[:, jt, :],
                        in_=cons_d[0:1, VPAD_OFF:VPAD_OFF + 520]
                        .to_broadcast([128, 520]))

                def qk_proj(p, wqk_t):
                    # k first (scores need all of k, only half of q at a time)
                    for src in (1, 0):          # 1=k, 0=q
                        for ic in range(4):
                            pj = ps.tile([128, 512], F32, tag="pj", bufs=2)
                            for kt in range(8):
                                nc.tensor.matmul(
                                    out=pj,
                                    lhsT=wqk_t[:, kt, src * 128:(src + 1) * 128],
                                    rhs=x_s[:, kt, ic * 512:(ic + 1) * 512],
                                    start=(kt == 0), stop=(kt == 7))
                            nc.vector.tensor_scalar_add(
                                qk_s[:, p % 2, src, ic * 512:(ic + 1) * 512],
                                pj, bqk_s[:, 4 * src + p, 0:1])

                def v_proj(jt):
                    pj = ps.tile([128, 512], F32, tag="pj", bufs=2)
                    for kt in range(8):
                        nc.tensor.matmul(
                            out=pj, lhsT=x_s[:, kt, jt * 128:(jt + 1) * 128],
                            rhs=wv_s[:, kt, :],
                            start=(kt == 0), stop=False)
                    nc.tensor.matmul(
                        out=pj, lhsT=cons_s[0:1, ONES_OFF:ONES_OFF + 128],
                        rhs=cons_s[0:1, BV_OFF:BV_OFF + 512],
                        start=False, stop=True)
                    nc.vector.tensor_copy(
                        v1_s[:, jt, :].rearrange("p (h c) -> p h c", c=65)[:, :, 0:64],
                        pj[:, :].rearrange("p (h c) -> p h c", c=64))

                def scores_exp(h, qh):
                    p, hl = h // 2, h % 2
                    part = slice(hl * 64, hl * 64 + 64)
                    eb = e_s[(2 * h + qh) % 3]
                    for jt in range(16):
                        st = ps.tile([128, 1024], F32, tag="s", bufs=2)
                        for i2 in range(2):
                            q0 = qh * 1024 + i2 * 512
                            nc.tensor.matmul(
                                out=st[:, i2 * 512:(i2 + 1) * 512],
                                lhsT=qk_s[part, p % 2, 1, jt * 128:(jt + 1) * 128],
                                rhs=qk_s[part, p % 2, 0, q0:q0 + 512],
                                start=True, stop=True)
                        nc.scalar.activation(out=eb[:, jt, :], in_=st, func=EXP,
                                             scale=SCALE)

                def pv_half(h, qh):
                    p, hl = h // 2, h % 2
                    eb = e_s[(2 * h + qh) % 3]
                    for qb in range(8):
                        pv = ps.tile([128, 512], F32, tag="pv", bufs=2)
                        for jt in range(16):
                            nc.tensor.matmul(
                                out=pv[:, 0:65],
                                lhsT=eb[:, jt, qb * 128:(qb + 1) * 128],
                                rhs=v1_s[:, jt, h * 65:h * 65 + 65],
                                start=(jt == 0), stop=(jt == 15))
                        rcp = pa.tile([128, 1], F32, tag="rcp", bufs=4)
                        nc.vector.reciprocal(rcp, pv[:, 64:65])
                        nc.vector.tensor_scalar_mul(
                            ctq_s[:, p % 2, qh * 8 + qb, hl * 64:hl * 64 + 64],
                            pv[:, 0:64], rcp)

                def transposes(p, qbs):
                    for qb in qbs:
                        nc.sync.dma_start_transpose(
                            out=ctx_s[:, p, qb * 128:(qb + 1) * 128],
                            in_=ctq_s[:, p % 2, qb, :])

                # ---- pair 0 warm-up
                wqk_t = wqk0_t
                qk_proj(0, wqk_t)
                scores_exp(0, 0)
                scores_exp(0, 1)
                for jt in range(16):     # v-proj overlaps head-0 exps
                    v_proj(jt)
                pv_half(0, 0)
                pv_half(0, 1)

                for p in range(4):
                    if p > 0:
                        qk_proj(p, wqk_t)
                        scores_exp(2 * p, 0)
                        scores_exp(2 * p, 1)
                        pv_half(2 * p, 0)
                        pv_half(2 * p, 1)
                    if p < 3:            # prefetch next pair's weights
                        wqk_t = px.tile([128, 8, 256], BF16, tag="wqk", bufs=2)
                        for kt in range(8):
                            nc.sync.dma_start(
                                out=wqk_t[:, kt, :],
                                in_=xw_d[kt * 128:(kt + 1) * 128,
                                         (p + 1) * 256:(p + 2) * 256])
                    h = 2 * p + 1
                    scores_exp(h, 0)
                    scores_exp(h, 1)
                    pv_half(h, 0)
                    if h == 7:
                        break            # tail handled below
                    pv_half(h, 1)
                    transposes(p, range(16))

              # px closed: x/wv/wqk SBUF freed for wo/ot
              with tc.tile_pool(name="pl", bufs=1) as pl:
                wo_s = pl.tile([128, 4, E], BF16)
                for ct in range(4):
                    nc.gpsimd.dma_start(
                        out=wo_s[:, ct, :],
                        in_=wo_d[ct * 128:(ct + 1) * 128, :])

                def out_proj(i2):
                    for et in range(8):
                        po = ps.tile([128, 512], F32, tag="pj", bufs=2)
                        for ct in range(4):
                            nc.tensor.matmul(
                                out=po, lhsT=wo_s[:, ct, et * 128:(et + 1) * 128],
                                rhs=ctx_s[:, ct, i2 * 512:(i2 + 1) * 512],
                                start=(ct == 0), stop=(ct == 3))
                        ot = pl.tile([128, 512], F32, tag="ot", bufs=4)
                        nc.vector.tensor_scalar_add(ot, po, bout_s[:, et, 0:1])
                        nc.sync.dma_start(
                            out=out_d[et * 128:(et + 1) * 128,
                                      i2 * 512:(i2 + 1) * 512],
                            in_=ot)

                transposes(3, range(8))
                out_proj(0)              # overlaps exp(7, qh1)
                out_proj(1)
                pv_half(7, 1)
                transposes(3, range(8, 16))
                out_proj(2)
                out_proj(3)
    nc.compile()
    return nc


_NC = None


def _get_nc():
    global _NC
    if _NC is None:
        _NC = build_nc()
    return _NC


def make_in_maps(query, Wqkv, bqkv, Wout, bout):
    query = np.asarray(query, dtype=np.float32)
    Wqkv = np.asarray(Wqkv, dtype=np.float32)
    bqkv = np.asarray(bqkv, dtype=np.float32)
    Wout = np.asarray(Wout, dtype=np.float32)
    bout = np.asarray(bout, dtype=np.float32)
    bf = ml_dtypes.bfloat16

    in_maps = []
    for c in range(8):
        b, hh = c // 2, c % 2
        heads = np.arange(hh * HL, hh * HL + HL)
        dims = (heads[:, None] * HD + np.arange(HD)[None, :]).reshape(-1)  # [512]
        q_rows, k_rows, v_rows = dims, E + dims, 2 * E + dims

        xw = np.empty((E, 3584), bf)
        for p in range(4):
            pd = dims[p * 128:(p + 1) * 128]
            xw[:, p * 256:p * 256 + 128] = Wqkv[pd].T.astype(bf)
            xw[:, p * 256 + 128:p * 256 + 256] = Wqkv[E + pd].T.astype(bf)
        xw[:, 1024:3072] = query[b].T.astype(bf)
        xw[:, 3072:3584] = Wqkv[v_rows].T.astype(bf)

        # bqk rows: [q pair0..3 | k pair0..3], each pair-major 128 rows
        bqk = np.concatenate([bqkv[q_rows], bqkv[k_rows]]).reshape(E, 1)

        cons = np.zeros((1, CONS_LEN), bf)
        cons[0, ONES_OFF:ONES_OFF + 128] = 1.0
        cons[0, BV_OFF:BV_OFF + 512] = bqkv[v_rows].astype(bf)
        vpad = np.zeros(520, bf)
        for i in range(8):
            vpad[i * 65 + 64] = 1.0
        cons[0, VPAD_OFF:VPAD_OFF + 520] = vpad

        wo = np.ascontiguousarray(Wout[:, dims].T).astype(bf)   # [512, E]
        bo = (bout if hh == 0 else np.zeros_like(bout)).reshape(E, 1)

        in_maps.append({
            "xw": xw, "bqk": np.ascontiguousarray(bqk),
            "cons": cons, "wo": wo, "bout": np.ascontiguousarray(bo),
        })
    return in_maps


def gather(results):
    out = np.empty((B, S, E), np.float32)
    for b in range(B):
        acc = results[2 * b]["outT"] + results[2 * b + 1]["outT"]   # [E, S]
        out[b] = acc.T
    return out


def kernel(query, key, value, Wqkv, bqkv, Wout, bout):
    # key/value are unused by the reference module (qkv all from query)
    nc = _get_nc()
    in_maps = make_in_maps(query, Wqkv, bqkv, Wout, bout)
    res = run_bass_kernel_spmd(nc, in_maps, list(range(8)))
    return gather(res.results)


# revision 31
# speedup vs baseline: 1.3108x; 1.1231x over previous
"""Multi-head attention (B=4, S=2048, E=1024, H=16) on 8 trn2 NeuronCores.

Sharding: data-parallel over B (4) x tensor-parallel over H (2 halves of 8
heads). Core c handles batch c//2, head-half c%2. Column-parallel qkv_proj,
row-parallel out_proj; the all-reduce of the two partial outputs per batch is
done on the host during unshard (a sum of two arrays), as is the final
transpose (the device emits out^T to keep DMA writes contiguous).

Device kernel v2 (bf16 matmuls, fp32 psum): per head-pair p, JIT qk-proj
(bf16, out evicted bf16); per head: scores^T per key-tile in [128 keys, 1024
queries] psum tiles, ACT exp -> e bf16 (scale 1/sqrt(E) folded); PV runs
TRANSPOSED: stationary = e-tile [128 keys, 128 queries], moving = [v_h | 1]
bf16 [128, 65], so psum accumulates [128 q, 64 ctx | softmax-denominator].
Normalization is then a per-partition reciprocal + tensor_scalar_mul (no
DRAM-bounce broadcast). ctx [q, d] tiles are transposed to [d, q] with the
DMA xbar (dma_start_transpose, off the PE critical path), then row-parallel
out-proj emits out^T partials. v-proj overlaps head-0's exps; out-proj's
first query-half overlaps the last head's second-half exps.
"""
import sys

import numpy as np

sys.path.insert(0, "/opt/trn_rl_repo")

import ml_dtypes

import concourse.bacc as bacc
import concourse.mybir as mybir
import concourse.tile as tile
from concourse.bass_utils import run_bass_kernel_spmd

F32 = mybir.dt.float32
BF16 = mybir.dt.bfloat16
EXP = mybir.ActivationFunctionType.Exp

B, S, E, H, HD = 4, 2048, 1024, 16, 64
HL = 8            # heads per core
SCALE = float(1.0 / np.sqrt(np.float32(E)))

# cons layout (bf16 row): [0:128] ones, [128:640] bv, [640:1160] v1 pad row
ONES_OFF, BV_OFF, VPAD_OFF, CONS_LEN = 0, 128, 640, 1160


def build_nc():
    nc = bacc.Bacc("TRN2", target_bir_lowering=False, debug=False, num_devices=8)
    # xw cols: [0:1024] per-pair [Wq_p^T | Wk_p^T] (4 x 256), [1024:3072] x^T,
    # [3072:3584] Wv_loc^T -- all bf16
    xw_d = nc.declare_dram_parameter("xw", [E, 3584], BF16, isOutput=False)
    bqk_d = nc.declare_dram_parameter("bqk", [E, 1], F32, isOutput=False)
    cons_d = nc.declare_dram_parameter("cons", [1, CONS_LEN], BF16, isOutput=False)
    wo_d = nc.declare_dram_parameter("wo", [512, E], BF16, isOutput=False)
    bout_d = nc.declare_dram_parameter("bout", [E, 1], F32, isOutput=False)
    out_d = nc.declare_dram_parameter("outT", [E, S], F32, isOutput=True)

    with tile.TileContext(nc) as tc:
      with tc.tile_pool(name="pp", bufs=1) as pp:
        bqk_s = pp.tile([128, 8, 1], F32)
        bout_s = pp.tile([128, 8, 1], F32)
        cons_s = pp.tile([1, CONS_LEN], BF16)
        nc.gpsimd.dma_start(out=cons_s, in_=cons_d[:, :])
        nc.gpsimd.dma_start(out=bqk_s, in_=bqk_d[:, :].rearrange("(m p) o -> p m o", p=128))
        nc.gpsimd.dma_start(out=bout_s, in_=bout_d[:, :].rearrange("(m p) o -> p m o", p=128))
        # warm the ACT exp table (load is ~1.3us; keep it off the critical path)
        warm = pp.tile([1, 1], F32)
        nc.scalar.activation(out=warm, in_=bqk_s[0:1, 0, 0:1], func=EXP)

        with tc.tile_pool(name="pa", bufs=1) as pa:
            # persistent phase-3 state
            qk_s = pa.tile([128, 2, 2, S], BF16)        # [slot, q|k, tok]
            v1_s = pa.tile([128, 16, 520], BF16)        # per jt: 8 x [v_h(64) | 1]
            e_s = [pa.tile([128, 16, 1024], BF16, name=f"e{i}") for i in range(3)]
            ctq_s = pa.tile([128, 2, 16, 128], BF16)    # [slot, qb, d-pair]
            ctx_s = pa.tile([128, 4, S], BF16)          # ctx^T per pair [d, q]

            with tc.tile_pool(name="ps", bufs=1, space="PSUM") as ps:
              with tc.tile_pool(name="px", bufs=1) as px:
                x_s = px.tile([128, 8, S], BF16)
                wv_s = px.tile([128, 8, 512], BF16)
                wqk0_t = px.tile([128, 8, 256], BF16, tag="wqk", bufs=2)
                for kt in range(8):
                    nc.gpsimd.dma_start(
                        out=wqk0_t[:, kt, :],
                        in_=xw_d[kt * 128:(kt + 1) * 128, 0:256])
                # ic-major, alternating queues: halves the per-queue DMA
                # issue serialization on the warm-up critical path
                for ic in range(4):
                    for kt in range(8):
                        eng = nc.sync if kt % 2 == 0 else nc.gpsimd
                        eng.dma_start(
                            out=x_s[:, kt, ic * 512:(ic + 1) * 512],
                            in_=xw_d[kt * 128:(kt + 1) * 128,
                                     1024 + ic * 512:1024 + (ic + 1) * 512])
                # after x on the sync queue: keeps the shared DMA engines
                # clear for the critical-path x load (wv/v1 needed ~30us in)
                for kt in range(8):
                    nc.sync.dma_start(
                        out=wv_s[:, kt, :],
                        in_=xw_d[kt * 128:(kt + 1) * 128, 3072:3584])
                for jt in range(16):
                    nc.sync.dma_start(
                        out=v1_s[:, jt, :],
                        in_=cons_d[0:1, VPAD_OFF:VPAD_OFF + 520]
                        .to_broadcast([128, 520]))

                def qk_proj_part(p, wqk_t, chunks):
                    for src, ic in chunks:      # src: 1=k, 0=q; ic: 512 tok
                        pj = ps.tile([128, 512], F32, tag="w", bufs=2)
                        for kt in range(8):
                            nc.tensor.matmul(
                                out=pj,
                                lhsT=wqk_t[:, kt, src * 128:(src + 1) * 128],
                                rhs=x_s[:, kt, ic * 512:(ic + 1) * 512],
                                start=(kt == 0), stop=(kt == 7))
                        nc.vector.tensor_scalar_add(
                            qk_s[:, p % 2, src, ic * 512:(ic + 1) * 512],
                            pj, bqk_s[:, 4 * src + p, 0:1])

                def qk_proj(p, wqk_t):
                    # k first (scores need all of k, only half of q at a time)
                    qk_proj_part(p, wqk_t,
                                 [(1, ic) for ic in range(4)]
                                 + [(0, ic) for ic in range(4)])

                def v_proj(g, jts):
                    # one head-pair's v columns (g = pair index), so the work
                    # spreads across the schedule instead of lumping 34us
                    for jt in jts:
                        pj = ps.tile([128, 512], F32, tag="w", bufs=2)
                        for kt in range(8):
                            nc.tensor.matmul(
                                out=pj[:, 0:128],
                                lhsT=x_s[:, kt, jt * 128:(jt + 1) * 128],
                                rhs=wv_s[:, kt, g * 128:(g + 1) * 128],
                                start=(kt == 0), stop=False)
                        nc.tensor.matmul(
                            out=pj[:, 0:128],
                            lhsT=cons_s[0:1, ONES_OFF:ONES_OFF + 128],
                            rhs=cons_s[0:1, BV_OFF + g * 128:BV_OFF + (g + 1) * 128],
                            start=False, stop=True)
                        nc.vector.tensor_copy(
                            v1_s[:, jt, g * 130:(g + 1) * 130]
                            .rearrange("p (h c) -> p h c", c=65)[:, :, 0:64],
                            pj[:, 0:128].rearrange("p (h c) -> p h c", c=64))

                def scores_exp(h, qh, i2s=(0, 1)):
                    # s-tiles pack 3 key-tiles x 512 queries: 1536-wide exps
                    # amortize ACT access overhead; i2-granular so PV can
                    # chase by 512-query sub-halves
                    p, hl = h // 2, h % 2
                    part = slice(hl * 64, hl * 64 + 64)
                    eb = e_s[(2 * h + qh) % 3]
                    for i2 in i2s:
                        q0 = qh * 1024 + i2 * 512
                        for jtg in range(6):       # jt triples, last has 1
                            j0, j1 = 3 * jtg, min(3 * jtg + 3, 16)
                            st = ps.tile([128, 3, 512], F32, tag="s", bufs=2)
                            for j in range(j0, j1):
                                nc.tensor.matmul(
                                    out=st[:, j - j0, :],
                                    lhsT=qk_s[part, p % 2, 1, j * 128:(j + 1) * 128],
                                    rhs=qk_s[part, p % 2, 0, q0:q0 + 512],
                                    start=True, stop=True)
                            nc.scalar.activation(
                                out=eb[:, j0:j1, i2 * 512:(i2 + 1) * 512],
                                in_=st[:, 0:j1 - j0, :],
                                func=EXP, scale=SCALE)

                def pv_part(h, qh, qbs):
                    # two query-blocks share one psum slot; one batched
                    # reciprocal per pair keeps DVE ahead of the PE
                    p, hl = h // 2, h % 2
                    eb = e_s[(2 * h + qh) % 3]
                    qbs = list(qbs)
                    for qq in range(0, len(qbs), 2):
                        pv = ps.tile([128, 2, 256], F32, tag="w", bufs=2)
                        for j2, qb in enumerate(qbs[qq:qq + 2]):
                            for jt in range(16):
                                nc.tensor.matmul(
                                    out=pv[:, j2, 0:65],
                                    lhsT=eb[:, jt, qb * 128:(qb + 1) * 128],
                                    rhs=v1_s[:, jt, h * 65:h * 65 + 65],
                                    start=(jt == 0), stop=(jt == 15))
                        rcp = pa.tile([128, 2], F32, tag="rcp", bufs=4)
                        nc.vector.reciprocal(rcp, pv[:, :, 64])
                        for j2, qb in enumerate(qbs[qq:qq + 2]):
                            nc.vector.tensor_scalar_mul(
                                ctq_s[:, p % 2, qh * 8 + qb, hl * 64:hl * 64 + 64],
                                pv[:, j2, 0:64], rcp[:, j2:j2 + 1])

                def pv_half(h, qh):
                    pv_part(h, qh, range(8))

                def transposes(p, qbs):
                    for qb in qbs:
                        nc.sync.dma_start_transpose(
                            out=ctx_s[:, p, qb * 128:(qb + 1) * 128],
                            in_=ctq_s[:, p % 2, qb, :])

                def wqk_load(p):
                    t = px.tile([128, 8, 256], BF16, tag="wqk", bufs=2)
                    for kt in range(8):
                        nc.sync.dma_start(
                            out=t[:, kt, :],
                            in_=xw_d[kt * 128:(kt + 1) * 128,
                                     p * 256:(p + 1) * 256])
                    return t

                def pe_warm(n):
                    # keep the PE busy (and its p-state hot) while x loads
                    for _ in range(n):
                        st = ps.tile([128, 3, 512], F32, tag="s", bufs=2)
                        nc.tensor.matmul(
                            out=st[:, 0, :],
                            lhsT=cons_s[0:1, 0:128], rhs=cons_s[0:1, 0:512],
                            start=True, stop=True)

                # ---- warm-up: pair-0 qk (k first, then q by halves so the
                # first scores start as early as possible); pair-0's v columns
                # fill the PE while ACT chews head-0 exps.
                pe_warm(10)
                for ic in range(4):
                    qk_proj_part(0, wqk0_t, [(1, ic)])
                    if ic < 3:
                        pe_warm(3)
                qk_proj_part(0, wqk0_t, [(0, 0), (0, 1)])
                scores_exp(0, 0)
                qk_proj_part(0, wqk0_t, [(0, 2), (0, 3)])
                wqk_t = wqk_load(1)
                scores_exp(0, 1)
                v_proj(0, range(16))

                # ---- steady state half-steps: pv of step k-2 (deps ready),
                # scores of step k (keeps ACT fed), then filler PE work packed
                # as late as its deadline allows (late heads have no fillers
                # left, so deadline-packing minimizes end-game PE idle).
                for k in range(2, 14):
                    h2, qh2 = (k - 2) // 2, (k - 2) % 2
                    h, qh = k // 2, k % 2
                    pv_half(h2, qh2)
                    scores_exp(h, qh)
                    q = k // 4 + 1                 # pair whose qk is due
                    g = k // 4                     # pair whose v cols are due
                    if k % 4 == 2 and q <= 3:
                        qk_proj_part(q, wqk_t, [(1, ic) for ic in range(4)])
                    elif k % 4 == 3 and q <= 3:
                        qk_proj_part(q, wqk_t, [(0, ic) for ic in range(4)])
                    elif k % 4 == 0:
                        if g <= 3:
                            v_proj(g, range(8))
                        if k == 4:
                            wqk_t = wqk_load(2)
                        if k == 8:
                            wqk_t = wqk_load(3)
                    elif k % 4 == 1:
                        if g <= 3:
                            v_proj(g, range(8, 16))
                        if k > 4:                  # pair k//4-1 ctx done
                            transposes(k // 4 - 1, range(16))
                pv_half(6, 0)
                scores_exp(7, 0)
                pv_half(6, 1)

              # px closed: x/wv/wqk SBUF freed for wo/ot
              with tc.tile_pool(name="pl", bufs=1) as pl:
                wo_s = pl.tile([128, 4, E], BF16)
                for ct in range(4):
                    nc.gpsimd.dma_start(
                        out=wo_s[:, ct, :],
                        in_=wo_d[ct * 128:(ct + 1) * 128, :])

                def out_proj(i2):
                    for et in range(8):
                        po = ps.tile([128, 512], F32, tag="w", bufs=2)
                        for ct in range(4):
                            nc.tensor.matmul(
                                out=po,
                                lhsT=wo_s[:, ct, et * 128:(et + 1) * 128],
                                rhs=ctx_s[:, ct, i2 * 512:(i2 + 1) * 512],
                                start=(ct == 0), stop=(ct == 3))
                        ot = pl.tile([128, 512], F32, tag="ot", bufs=4)
                        nc.vector.tensor_scalar_add(ot, po, bout_s[:, et, 0:1])
                        nc.sync.dma_start(
                            out=out_d[et * 128:(et + 1) * 128,
                                      i2 * 512:(i2 + 1) * 512],
                            in_=ot)

                # tail: weave the last scores half with the pv/out-proj
                # chase at sub-half granularity
                scores_exp(7, 1, i2s=(0,))
                pv_part(7, 0, range(4))
                transposes(3, range(4))
                out_proj(0)
                scores_exp(7, 1, i2s=(1,))
                pv_part(7, 0, range(4, 8))
                transposes(3, range(4, 8))
                out_proj(1)
                pv_part(7, 1, range(4))
                transposes(3, range(8, 12))
                out_proj(2)
                pv_part(7, 1, range(4, 8))
                transposes(3, range(12, 16))
                out_proj(3)
    nc.compile()
    return nc


_NC = None


def _get_nc():
    global _NC
    if _NC is None:
        _NC = build_nc()
    return _NC


def make_in_maps(query, Wqkv, bqkv, Wout, bout):
    query = np.asarray(query, dtype=np.float32)
    Wqkv = np.asarray(Wqkv, dtype=np.float32)
    bqkv = np.asarray(bqkv, dtype=np.float32)
    Wout = np.asarray(Wout, dtype=np.float32)
    bout = np.asarray(bout, dtype=np.float32)
    bf = ml_dtypes.bfloat16

    in_maps = []
    for c in range(8):
        b, hh = c // 2, c % 2
        heads = np.arange(hh * HL, hh * HL + HL)
        dims = (heads[:, None] * HD + np.arange(HD)[None, :]).reshape(-1)  # [512]
        q_rows, k_rows, v_rows = dims, E + dims, 2 * E + dims

        xw = np.empty((E, 3584), bf)
        for p in range(4):
            pd = dims[p * 128:(p + 1) * 128]
            xw[:, p * 256:p * 256 + 128] = Wqkv[pd].T.astype(bf)
            xw[:, p * 256 + 128:p * 256 + 256] = Wqkv[E + pd].T.astype(bf)
        xw[:, 1024:3072] = query[b].T.astype(bf)
        xw[:, 3072:3584] = Wqkv[v_rows].T.astype(bf)

        # bqk rows: [q pair0..3 | k pair0..3], each pair-major 128 rows
        bqk = np.concatenate([bqkv[q_rows], bqkv[k_rows]]).reshape(E, 1)

        cons = np.zeros((1, CONS_LEN), bf)
        cons[0, ONES_OFF:ONES_OFF + 128] = 1.0
        cons[0, BV_OFF:BV_OFF + 512] = bqkv[v_rows].astype(bf)
        vpad = np.zeros(520, bf)
        for i in range(8):
            vpad[i * 65 + 64] = 1.0
        cons[0, VPAD_OFF:VPAD_OFF + 520] = vpad

        wo = np.ascontiguousarray(Wout[:, dims].T).astype(bf)   # [512, E]
        bo = (bout if hh == 0 else np.zeros_like(bout)).reshape(E, 1)

        in_maps.append({
            "xw": xw, "bqk": np.ascontiguousarray(bqk),
            "cons": cons, "wo": wo, "bout": np.ascontiguousarray(bo),
        })
    return in_maps


def gather(results):
    out = np.empty((B, S, E), np.float32)
    for b in range(B):
        acc = results[2 * b]["outT"] + results[2 * b + 1]["outT"]   # [E, S]
        out[b] = acc.T
    return out


def kernel(query, key, value, Wqkv, bqkv, Wout, bout):
    # key/value are unused by the reference module (qkv all from query)
    nc = _get_nc()
    in_maps = make_in_maps(query, Wqkv, bqkv, Wout, bout)
    res = run_bass_kernel_spmd(nc, in_maps, list(range(8)))
    return gather(res.results)


# revision 35
# speedup vs baseline: 1.3144x; 1.0027x over previous
"""Multi-head attention (B=4, S=2048, E=1024, H=16) on 8 trn2 NeuronCores.

Sharding: data-parallel over B (4) x tensor-parallel over H (2 halves of 8
heads). Core c handles batch c//2, head-half c%2. Column-parallel qkv_proj,
row-parallel out_proj; the all-reduce of the two partial outputs per batch is
done on the host during unshard (a sum of two arrays), as is the final
transpose (the device emits out^T to keep DMA writes contiguous).

Device kernel v2 (bf16 matmuls, fp32 psum): per head-pair p, JIT qk-proj
(bf16, out evicted bf16); per head: scores^T per key-tile in [128 keys, 1024
queries] psum tiles, ACT exp -> e bf16 (scale 1/sqrt(E) folded); PV runs
TRANSPOSED: stationary = e-tile [128 keys, 128 queries], moving = [v_h | 1]
bf16 [128, 65], so psum accumulates [128 q, 64 ctx | softmax-denominator].
Normalization is then a per-partition reciprocal + tensor_scalar_mul (no
DRAM-bounce broadcast). ctx [q, d] tiles are transposed to [d, q] with the
DMA xbar (dma_start_transpose, off the PE critical path), then row-parallel
out-proj emits out^T partials. v-proj overlaps head-0's exps; out-proj's
first query-half overlaps the last head's second-half exps.
"""
import sys

import numpy as np

sys.path.insert(0, "/opt/trn_rl_repo")

import ml_dtypes

import concourse.bacc as bacc
import concourse.mybir as mybir
import concourse.tile as tile
from concourse.bass_utils import run_bass_kernel_spmd

F32 = mybir.dt.float32
BF16 = mybir.dt.bfloat16
EXP = mybir.ActivationFunctionType.Exp

B, S, E, H, HD = 4, 2048, 1024, 16, 64
HL = 8            # heads per core
SCALE = float(1.0 / np.sqrt(np.float32(E)))

# cons layout (bf16 row): [0:128] ones, [128:640] bv, [640:1160] v1 pad row
ONES_OFF, BV_OFF, VPAD_OFF, CONS_LEN = 0, 128, 640, 1160


def build_nc():
    nc = bacc.Bacc("TRN2", target_bir_lowering=False, debug=False, num_devices=8)
    # xw cols: [0:1024] per-pair [Wq_p^T | Wk_p^T] (4 x 256), [1024:3072] x^T,
    # [3072:3584] Wv_loc^T -- all bf16
    xw_d = nc.declare_dram_parameter("xw", [E, 3584], BF16, isOutput=False)
    bqk_d = nc.declare_dram_parameter("bqk", [E, 1], F32, isOutput=False)
    cons_d = nc.declare_dram_parameter("cons", [1, CONS_LEN], BF16, isOutput=False)
    wo_d = nc.declare_dram_parameter("wo", [512, E], BF16, isOutput=False)
    bout_d = nc.declare_dram_parameter("bout", [E, 1], F32, isOutput=False)
    out_d = nc.declare_dram_parameter("outT", [E, S], F32, isOutput=True)

    with tile.TileContext(nc) as tc:
      with tc.tile_pool(name="pp", bufs=1) as pp:
        bqk_s = pp.tile([128, 8, 1], F32)
        bout_s = pp.tile([128, 8, 1], F32)
        cons_s = pp.tile([1, CONS_LEN], BF16)
        nc.gpsimd.dma_start(out=cons_s, in_=cons_d[:, :])
        nc.gpsimd.dma_start(out=bqk_s, in_=bqk_d[:, :].rearrange("(m p) o -> p m o", p=128))
        nc.gpsimd.dma_start(out=bout_s, in_=bout_d[:, :].rearrange("(m p) o -> p m o", p=128))
        # warm the ACT exp table (load is ~1.3us; keep it off the critical path)
        warm = pp.tile([1, 1], F32)
        nc.scalar.activation(out=warm, in_=bqk_s[0:1, 0, 0:1], func=EXP)

        with tc.tile_pool(name="pa", bufs=1) as pa:
            # persistent phase-3 state
            qk_s = pa.tile([128, 2, 2, S], BF16)        # [slot, q|k, tok]
            v1_s = pa.tile([128, 16, 520], BF16)        # per jt: 8 x [v_h(64) | 1]
            e_s = [pa.tile([128, 16, 1024], BF16, name=f"e{i}") for i in range(3)]
            ctq_s = pa.tile([128, 2, 16, 128], BF16)    # [slot, qb, d-pair]
            ctx_s = pa.tile([128, 4, S], BF16)          # ctx^T per pair [d, q]

            with tc.tile_pool(name="ps", bufs=1, space="PSUM") as ps:
              with tc.tile_pool(name="px", bufs=1) as px:
                x_s = px.tile([128, 8, S], BF16)
                wv_s = px.tile([128, 8, 512], BF16)
                wqk0_t = px.tile([128, 8, 256], BF16, tag="wqk", bufs=2)
                for kt in range(8):
                    nc.gpsimd.dma_start(
                        out=wqk0_t[:, kt, :],
                        in_=xw_d[kt * 128:(kt + 1) * 128, 0:256])
                # ic-major, alternating queues: halves the per-queue DMA
                # issue serialization on the warm-up critical path
                for ic in range(4):
                    for kt in range(8):
                        eng = nc.sync if kt % 2 == 0 else nc.gpsimd
                        eng.dma_start(
                            out=x_s[:, kt, ic * 512:(ic + 1) * 512],
                            in_=xw_d[kt * 128:(kt + 1) * 128,
                                     1024 + ic * 512:1024 + (ic + 1) * 512])
                # after x on the sync queue: keeps the shared DMA engines
                # clear for the critical-path x load (wv/v1 needed ~30us in)
                for kt in range(8):
                    nc.sync.dma_start(
                        out=wv_s[:, kt, :],
                        in_=xw_d[kt * 128:(kt + 1) * 128, 3072:3584])
                for jt in range(16):
                    nc.sync.dma_start(
                        out=v1_s[:, jt, :],
                        in_=cons_d[0:1, VPAD_OFF:VPAD_OFF + 520]
                        .to_broadcast([128, 520]))

                def qk_proj_part(p, wqk_t, chunks):
                    for src, ic in chunks:      # src: 1=k, 0=q; ic: 512 tok
                        pj = ps.tile([128, 512], F32, tag="w", bufs=2)
                        for kt in range(8):
                            nc.tensor.matmul(
                                out=pj,
                                lhsT=wqk_t[:, kt, src * 128:(src + 1) * 128],
                                rhs=x_s[:, kt, ic * 512:(ic + 1) * 512],
                                start=(kt == 0), stop=(kt == 7))
                        nc.vector.tensor_scalar_add(
                            qk_s[:, p % 2, src, ic * 512:(ic + 1) * 512],
                            pj, bqk_s[:, 4 * src + p, 0:1])

                def qk_proj(p, wqk_t):
                    # k first (scores need all of k, only half of q at a time)
                    qk_proj_part(p, wqk_t,
                                 [(1, ic) for ic in range(4)]
                                 + [(0, ic) for ic in range(4)])

                def v_proj(g, jts):
                    # one head-pair's v columns (g = pair index), so the work
                    # spreads across the schedule instead of lumping 34us
                    for jt in jts:
                        pj = ps.tile([128, 512], F32, tag="w", bufs=2)
                        for kt in range(8):
                            nc.tensor.matmul(
                                out=pj[:, 0:128],
                                lhsT=x_s[:, kt, jt * 128:(jt + 1) * 128],
                                rhs=wv_s[:, kt, g * 128:(g + 1) * 128],
                                start=(kt == 0), stop=False)
                        nc.tensor.matmul(
                            out=pj[:, 0:128],
                            lhsT=cons_s[0:1, ONES_OFF:ONES_OFF + 128],
                            rhs=cons_s[0:1, BV_OFF + g * 128:BV_OFF + (g + 1) * 128],
                            start=False, stop=True)
                        nc.vector.tensor_copy(
                            v1_s[:, jt, g * 130:(g + 1) * 130]
                            .rearrange("p (h c) -> p h c", c=65)[:, :, 0:64],
                            pj[:, 0:128].rearrange("p (h c) -> p h c", c=64))

                def scores_exp(h, qh, i2s=(0, 1)):
                    # s-tiles pack 3 key-tiles x 512 queries: 1536-wide exps
                    # amortize ACT access overhead; i2-granular so PV can
                    # chase by 512-query sub-halves
                    p, hl = h // 2, h % 2
                    part = slice(hl * 64, hl * 64 + 64)
                    eb = e_s[(2 * h + qh) % 3]
                    for i2 in i2s:
                        q0 = qh * 1024 + i2 * 512
                        for jtg in range(6):       # jt triples, last has 1
                            j0, j1 = 3 * jtg, min(3 * jtg + 3, 16)
                            st = ps.tile([128, 3, 512], F32, tag="s", bufs=2)
                            for j in range(j0, j1):
                                nc.tensor.matmul(
                                    out=st[:, j - j0, :],
                                    lhsT=qk_s[part, p % 2, 1, j * 128:(j + 1) * 128],
                                    rhs=qk_s[part, p % 2, 0, q0:q0 + 512],
                                    start=True, stop=True)
                            nc.scalar.activation(
                                out=eb[:, j0:j1, i2 * 512:(i2 + 1) * 512],
                                in_=st[:, 0:j1 - j0, :],
                                func=EXP, scale=SCALE)

                def pv_part(h, qh, qbs):
                    # two query-blocks share one psum slot; one batched
                    # reciprocal per pair keeps DVE ahead of the PE
                    p, hl = h // 2, h % 2
                    eb = e_s[(2 * h + qh) % 3]
                    qbs = list(qbs)
                    for qq in range(0, len(qbs), 2):
                        pv = ps.tile([128, 2, 256], F32, tag="w", bufs=2)
                        for j2, qb in enumerate(qbs[qq:qq + 2]):
                            for jt in range(16):
                                nc.tensor.matmul(
                                    out=pv[:, j2, 0:65],
                                    lhsT=eb[:, jt, qb * 128:(qb + 1) * 128],
                                    rhs=v1_s[:, jt, h * 65:h * 65 + 65],
                                    start=(jt == 0), stop=(jt == 15))
                        rcp = pa.tile([128, 2], F32, tag="rcp", bufs=4)
                        nc.vector.reciprocal(rcp, pv[:, :, 64])
                        for j2, qb in enumerate(qbs[qq:qq + 2]):
                            nc.vector.tensor_scalar_mul(
                                ctq_s[:, p % 2, qh * 8 + qb, hl * 64:hl * 64 + 64],
                                pv[:, j2, 0:64], rcp[:, j2:j2 + 1])

                def pv_half(h, qh):
                    pv_part(h, qh, range(8))

                def transposes(p, qbs):
                    for qb in qbs:
                        nc.sync.dma_start_transpose(
                            out=ctx_s[:, p, qb * 128:(qb + 1) * 128],
                            in_=ctq_s[:, p % 2, qb, :])

                def wqk_load(p):
                    t = px.tile([128, 8, 256], BF16, tag="wqk", bufs=2)
                    for kt in range(8):
                        nc.sync.dma_start(
                            out=t[:, kt, :],
                            in_=xw_d[kt * 128:(kt + 1) * 128,
                                     p * 256:(p + 1) * 256])
                    return t

                wz = px.tile([1, 640], BF16)
                nc.vector.memset(wz, 0.5)

                def pe_warm(n):
                    # keep the PE busy (and its p-state hot) while x loads
                    for _ in range(n):
                        st = ps.tile([128, 3, 512], F32, tag="s", bufs=2)
                        nc.tensor.matmul(
                            out=st[:, 0, :],
                            lhsT=wz[0:1, 0:128], rhs=wz[0:1, 0:512],
                            start=True, stop=True)

                # ---- warm-up: pair-0 qk (k first, then q by halves so the
                # first scores start as early as possible); pair-0's v columns
                # fill the PE while ACT chews head-0 exps.
                pe_warm(10)
                for ic in range(4):
                    qk_proj_part(0, wqk0_t, [(1, ic)])
                    if ic < 3:
                        pe_warm(3)
                qk_proj_part(0, wqk0_t, [(0, 0), (0, 1)])
                scores_exp(0, 0)
                qk_proj_part(0, wqk0_t, [(0, 2), (0, 3)])
                wqk_t = wqk_load(1)
                scores_exp(0, 1)
                v_proj(0, range(16))

                # ---- steady state half-steps: pv of step k-2 (deps ready),
                # scores of step k (keeps ACT fed), then filler PE work packed
                # as late as its deadline allows (late heads have no fillers
                # left, so deadline-packing minimizes end-game PE idle).
                for k in range(2, 14):
                    h2, qh2 = (k - 2) // 2, (k - 2) % 2
                    h, qh = k // 2, k % 2
                    pv_half(h2, qh2)
                    scores_exp(h, qh)
                    q = k // 4 + 1                 # pair whose qk is due
                    g = k // 4                     # pair whose v cols are due
                    if k % 4 == 2 and q <= 3:
                        qk_proj_part(q, wqk_t, [(1, ic) for ic in range(4)])
                    elif k % 4 == 3 and q <= 3:
                        qk_proj_part(q, wqk_t, [(0, ic) for ic in range(4)])
                    elif k % 4 == 0:
                        if g <= 3:
                            v_proj(g, range(8))
                        if k == 4:
                            wqk_t = wqk_load(2)
                        if k == 8:
                            wqk_t = wqk_load(3)
                    elif k % 4 == 1:
                        if g <= 3:
                            v_proj(g, range(8, 16))
                        if k > 4:                  # pair k//4-1 ctx done
                            transposes(k // 4 - 1, range(16))
                pv_half(6, 0)
                scores_exp(7, 0)
                pv_half(6, 1)

              # px closed: x/wv/wqk SBUF freed for wo/ot
              with tc.tile_pool(name="pl", bufs=1) as pl:
                wo_s = pl.tile([128, 4, E], BF16)
                for ct in range(4):
                    nc.gpsimd.dma_start(
                        out=wo_s[:, ct, :],
                        in_=wo_d[ct * 128:(ct + 1) * 128, :])

                def out_proj(i2):
                    for et in range(8):
                        po = ps.tile([128, 512], F32, tag="w", bufs=2)
                        for ct in range(4):
                            nc.tensor.matmul(
                                out=po,
                                lhsT=wo_s[:, ct, et * 128:(et + 1) * 128],
                                rhs=ctx_s[:, ct, i2 * 512:(i2 + 1) * 512],
                                start=(ct == 0), stop=(ct == 3))
                        ot = pl.tile([128, 512], F32, tag="ot", bufs=4)
                        nc.vector.tensor_scalar_add(ot, po, bout_s[:, et, 0:1])
                        nc.sync.dma_start(
                            out=out_d[et * 128:(et + 1) * 128,
                                      i2 * 512:(i2 + 1) * 512],
                            in_=ot)

                # tail: weave the last scores half with the pv/out-proj
                # chase at sub-half granularity
                scores_exp(7, 1, i2s=(0,))
                pv_part(7, 0, range(4))
                transposes(3, range(4))
                out_proj(0)
                scores_exp(7, 1, i2s=(1,))
                pv_part(7, 0, range(4, 8))
                transposes(3, range(4, 8))
                out_proj(1)
                pv_part(7, 1, range(4))
                transposes(3, range(8, 12))
                out_proj(2)
                pv_part(7, 1, range(4, 8))
                transposes(3, range(12, 16))
                out_proj(3)
    nc.compile()
    return nc


_NC = None


def _get_nc():
    global _NC
    if _NC is None:
        _NC = build_nc()
    return _NC


def make_in_maps(query, Wqkv, bqkv, Wout, bout):
    query = np.asarray(query, dtype=np.float32)
    Wqkv = np.asarray(Wqkv, dtype=np.float32)
    bqkv = np.asarray(bqkv, dtype=np.float32)
    Wout = np.asarray(Wout, dtype=np.float32)
    bout = np.asarray(bout, dtype=np.float32)
    bf = ml_dtypes.bfloat16

    in_maps = []
    for c in range(8):
        b, hh = c // 2, c % 2
        heads = np.arange(hh * HL, hh * HL + HL)
        dims = (heads[:, None] * HD + np.arange(HD)[None, :]).reshape(-1)  # [512]
        q_rows, k_rows, v_rows = dims, E + dims, 2 * E + dims

        xw = np.empty((E, 3584), bf)
        for p in range(4):
            pd = dims[p * 128:(p + 1) * 128]
            xw[:, p * 256:p * 256 + 128] = Wqkv[pd].T.astype(bf)
            xw[:, p * 256 + 128:p * 256 + 256] = Wqkv[E + pd].T.astype(bf)
        xw[:, 1024:3072] = query[b].T.astype(bf)
        xw[:, 3072:3584] = Wqkv[v_rows].T.astype(bf)

        # bqk rows: [q pair0..3 | k pair0..3], each pair-major 128 rows
        bqk = np.concatenate([bqkv[q_rows], bqkv[k_rows]]).reshape(E, 1)

        cons = np.zeros((1, CONS_LEN), bf)
        cons[0, ONES_OFF:ONES_OFF + 128] = 1.0
        cons[0, BV_OFF:BV_OFF + 512] = bqkv[v_rows].astype(bf)
        vpad = np.zeros(520, bf)
        for i in range(8):
            vpad[i * 65 + 64] = 1.0
        cons[0, VPAD_OFF:VPAD_OFF + 520] = vpad

        wo = np.ascontiguousarray(Wout[:, dims].T).astype(bf)   # [512, E]
        bo = (bout if hh == 0 else np.zeros_like(bout)).reshape(E, 1)

        in_maps.append({
            "xw": xw, "bqk": np.ascontiguousarray(bqk),
            "cons": cons, "wo": wo, "bout": np.ascontiguousarray(bo),
        })
    return in_maps


def gather(results):
    out = np.empty((B, S, E), np.float32)
    for b in range(B):
        acc = results[2 * b]["outT"] + results[2 * b + 1]["outT"]   # [E, S]
        out[b] = acc.T
    return out


def kernel(query, key, value, Wqkv, bqkv, Wout, bout):
    # key/value are unused by the reference module (qkv all from query)
    nc = _get_nc()
    in_maps = make_in_maps(query, Wqkv, bqkv, Wout, bout)
    res = run_bass_kernel_spmd(nc, in_maps, list(range(8)))
    return gather(res.results)


# revision 38
# speedup vs baseline: 1.3413x; 1.0205x over previous
"""Multi-head attention (B=4, S=2048, E=1024, H=16) on 8 trn2 NeuronCores.

Sharding: data-parallel over B (4) x tensor-parallel over H (2 halves of 8
heads). Core c handles batch c//2, head-half c%2. Column-parallel qkv_proj,
row-parallel out_proj; the all-reduce of the two partial outputs per batch is
done on the host during unshard (a sum of two arrays), as is the final
transpose (the device emits out^T to keep DMA writes contiguous).

Device kernel v2 (bf16 matmuls, fp32 psum), 306us vs the 403us fp32r
baseline. Key structure:
  - scores^T per key-tile-triple in [128 keys, 3 jt, 512 q] psum tiles; one
    1536-wide ACT exp per tile (amortizes ACT access overhead) -> e bf16
    with the 1/sqrt(E) scale folded in. ACT exp is ~255us busy, the #2
    engine after PE (~282us busy, 92%).
  - PV runs TRANSPOSED: stationary = e-tile [128 keys, 128 queries], moving
    = [v_h | 1] bf16 [128, 65], psum accumulates [128 q, 64 ctx | softmax
    denominator] - halves PV cycles vs the [65, q] form and makes the
    normalizer a per-partition scalar: reciprocal + tensor_scalar_mul on
    DVE, no DRAM-bounce partition broadcast.
  - ctx [q, d] -> [d, q] via the DMA xbar (dma_start_transpose), zero PE.
  - software-pipelined emission at half-head granularity: qk-proj (JIT per
    pair) and per-pair v-proj columns are deadline-packed as fillers so the
    PE never starves while ACT paces the exps; memset-fed dummy matmuls
    keep the PE p-state hot during the initial x load (split across the
    sync+gpsimd DMA queues); the last head runs 512-query-fine so PV /
    xbar-transpose / out-proj chase the final exps at sub-half granularity.
"""
import sys

import numpy as np

sys.path.insert(0, "/opt/trn_rl_repo")

import ml_dtypes

import concourse.bacc as bacc
import concourse.mybir as mybir
import concourse.tile as tile
from concourse.bass_utils import run_bass_kernel_spmd

F32 = mybir.dt.float32
BF16 = mybir.dt.bfloat16
EXP = mybir.ActivationFunctionType.Exp

B, S, E, H, HD = 4, 2048, 1024, 16, 64
HL = 8            # heads per core
SCALE = float(1.0 / np.sqrt(np.float32(E)))

# cons layout (bf16 row): [0:128] ones, [128:640] bv, [640:1160] v1 pad row
ONES_OFF, BV_OFF, VPAD_OFF, CONS_LEN = 0, 128, 640, 1160


def build_nc():
    nc = bacc.Bacc("TRN2", target_bir_lowering=False, debug=False, num_devices=8)
    # xw cols: [0:1024] per-pair [Wq_p^T | Wk_p^T] (4 x 256), [1024:3072] x^T,
    # [3072:3584] Wv_loc^T -- all bf16
    xw_d = nc.declare_dram_parameter("xw", [E, 3584], BF16, isOutput=False)
    bqk_d = nc.declare_dram_parameter("bqk", [E, 1], F32, isOutput=False)
    cons_d = nc.declare_dram_parameter("cons", [1, CONS_LEN], BF16, isOutput=False)
    wo_d = nc.declare_dram_parameter("wo", [512, E], BF16, isOutput=False)
    bout_d = nc.declare_dram_parameter("bout", [E, 1], F32, isOutput=False)
    out_d = nc.declare_dram_parameter("outT", [E, S], F32, isOutput=True)

    with tile.TileContext(nc) as tc:
      with tc.tile_pool(name="pp", bufs=1) as pp:
        bqk_s = pp.tile([128, 8, 1], F32)
        bout_s = pp.tile([128, 8, 1], F32)
        cons_s = pp.tile([1, CONS_LEN], BF16)
        nc.gpsimd.dma_start(out=cons_s, in_=cons_d[:, :])
        nc.gpsimd.dma_start(out=bqk_s, in_=bqk_d[:, :].rearrange("(m p) o -> p m o", p=128))
        nc.gpsimd.dma_start(out=bout_s, in_=bout_d[:, :].rearrange("(m p) o -> p m o", p=128))
        # warm the ACT exp table (load is ~1.3us; keep it off the critical path)
        warm = pp.tile([1, 1], F32)
        nc.scalar.activation(out=warm, in_=bqk_s[0:1, 0, 0:1], func=EXP)

        with tc.tile_pool(name="pa", bufs=1) as pa:
            # persistent phase-3 state
            qk_s = pa.tile([128, 2, 2, S], BF16)        # [slot, q|k, tok]
            v1_s = pa.tile([128, 16, 520], BF16)        # per jt: 8 x [v_h(64) | 1]
            e_s = [pa.tile([128, 16, 1024], BF16, name=f"e{i}") for i in range(3)]
            ctq_s = pa.tile([128, 2, 16, 128], BF16)    # [slot, qb, d-pair]
            ctx_s = pa.tile([128, 4, S], BF16)          # ctx^T per pair [d, q]

            with tc.tile_pool(name="ps", bufs=1, space="PSUM") as ps:
              with tc.tile_pool(name="px", bufs=1) as px:
                x_s = px.tile([128, 8, S], BF16)
                wv_s = px.tile([128, 8, 512], BF16)
                wqk0_t = px.tile([128, 8, 256], BF16, tag="wqk", bufs=2)
                for kt in range(8):
                    eng = nc.gpsimd if kt % 2 == 0 else nc.sync
                    eng.dma_start(
                        out=wqk0_t[:, kt, :],
                        in_=xw_d[kt * 128:(kt + 1) * 128, 0:256])
                # ic-major, alternating queues: halves the per-queue DMA
                # issue serialization on the warm-up critical path
                for ic in range(4):
                    for kt in range(8):
                        eng = nc.sync if kt % 2 == 0 else nc.gpsimd
                        eng.dma_start(
                            out=x_s[:, kt, ic * 512:(ic + 1) * 512],
                            in_=xw_d[kt * 128:(kt + 1) * 128,
                                     1024 + ic * 512:1024 + (ic + 1) * 512])
                # after x on the sync queue: keeps the shared DMA engines
                # clear for the critical-path x load (wv/v1 needed ~30us in)
                for kt in range(8):
                    nc.sync.dma_start(
                        out=wv_s[:, kt, :],
                        in_=xw_d[kt * 128:(kt + 1) * 128, 3072:3584])
                for jt in range(16):
                    nc.sync.dma_start(
                        out=v1_s[:, jt, :],
                        in_=cons_d[0:1, VPAD_OFF:VPAD_OFF + 520]
                        .to_broadcast([128, 520]))
                bv_s = px.tile([128, 512], BF16)
                nc.sync.dma_start(
                    out=bv_s,
                    in_=cons_d[0:1, BV_OFF:BV_OFF + 512].to_broadcast([128, 512]))

                def qk_proj_part(p, wqk_t, chunks):
                    for src, ic in chunks:      # src: 1=k, 0=q; ic: 512 tok
                        pj = ps.tile([128, 512], F32, tag="w", bufs=2)
                        for kt in range(8):
                            nc.tensor.matmul(
                                out=pj,
                                lhsT=wqk_t[:, kt, src * 128:(src + 1) * 128],
                                rhs=x_s[:, kt, ic * 512:(ic + 1) * 512],
                                start=(kt == 0), stop=(kt == 7))
                        nc.vector.tensor_scalar_add(
                            qk_s[:, p % 2, src, ic * 512:(ic + 1) * 512],
                            pj, bqk_s[:, 4 * src + p, 0:1])

                def qk_proj(p, wqk_t):
                    # k first (scores need all of k, only half of q at a time)
                    qk_proj_part(p, wqk_t,
                                 [(1, ic) for ic in range(4)]
                                 + [(0, ic) for ic in range(4)])

                def v_proj(g, jts):
                    # one head-pair's v columns (g = pair index), so the work
                    # spreads across the schedule instead of lumping 34us
                    for jt in jts:
                        pj = ps.tile([128, 512], F32, tag="w", bufs=2)
                        for kt in range(8):
                            nc.tensor.matmul(
                                out=pj[:, 0:128],
                                lhsT=x_s[:, kt, jt * 128:(jt + 1) * 128],
                                rhs=wv_s[:, kt, g * 128:(g + 1) * 128],
                                start=(kt == 0), stop=(kt == 7))
                        nc.vector.tensor_add(
                            v1_s[:, jt, g * 130:(g + 1) * 130]
                            .rearrange("p (h c) -> p h c", c=65)[:, :, 0:64],
                            pj[:, 0:128].rearrange("p (h c) -> p h c", c=64),
                            bv_s[:, g * 128:(g + 1) * 128]
                            .rearrange("p (h c) -> p h c", c=64))

                def scores_exp(h, qh, i2s=(0, 1), jtgs=tuple(range(6))):
                    # s-tiles pack 3 key-tiles x 512 queries: 1536-wide exps
                    # amortize ACT access overhead; i2-granular so PV can
                    # chase by 512-query sub-halves
                    p, hl = h // 2, h % 2
                    part = slice(hl * 64, hl * 64 + 64)
                    eb = e_s[(2 * h + qh) % 3]
                    for i2 in i2s:
                        q0 = qh * 1024 + i2 * 512
                        for jtg in jtgs:           # jt triples, last has 1
                            j0, j1 = 3 * jtg, min(3 * jtg + 3, 16)
                            st = ps.tile([128, 3, 512], F32, tag="s", bufs=2)
                            for j in range(j0, j1):
                                nc.tensor.matmul(
                                    out=st[:, j - j0, :],
                                    lhsT=qk_s[part, p % 2, 1, j * 128:(j + 1) * 128],
                                    rhs=qk_s[part, p % 2, 0, q0:q0 + 512],
                                    start=True, stop=True)
                            nc.scalar.activation(
                                out=eb[:, j0:j1, i2 * 512:(i2 + 1) * 512],
                                in_=st[:, 0:j1 - j0, :],
                                func=EXP, scale=SCALE)

                def pv_part(h, qh, qbs):
                    # two query-blocks share one psum slot; one batched
                    # reciprocal per pair keeps DVE ahead of the PE
                    p, hl = h // 2, h % 2
                    eb = e_s[(2 * h + qh) % 3]
                    qbs = list(qbs)
                    for qq in range(0, len(qbs), 2):
                        pv = ps.tile([128, 2, 256], F32, tag="w", bufs=2)
                        for j2, qb in enumerate(qbs[qq:qq + 2]):
                            for jt in range(16):
                                nc.tensor.matmul(
                                    out=pv[:, j2, 0:65],
                                    lhsT=eb[:, jt, qb * 128:(qb + 1) * 128],
                                    rhs=v1_s[:, jt, h * 65:h * 65 + 65],
                                    start=(jt == 0), stop=(jt == 15))
                        rcp = pa.tile([128, 2], F32, tag="rcp", bufs=4)
                        nc.vector.reciprocal(rcp, pv[:, :, 64])
                        for j2, qb in enumerate(qbs[qq:qq + 2]):
                            nc.vector.tensor_scalar_mul(
                                ctq_s[:, p % 2, qh * 8 + qb, hl * 64:hl * 64 + 64],
                                pv[:, j2, 0:64], rcp[:, j2:j2 + 1])

                def pv_half(h, qh):
                    pv_part(h, qh, range(8))

                def transposes(p, qbs):
                    for qb in qbs:
                        nc.sync.dma_start_transpose(
                            out=ctx_s[:, p, qb * 128:(qb + 1) * 128],
                            in_=ctq_s[:, p % 2, qb, :])

                def wqk_load(p):
                    t = px.tile([128, 8, 256], BF16, tag="wqk", bufs=2)
                    for kt in range(8):
                        nc.sync.dma_start(
                            out=t[:, kt, :],
                            in_=xw_d[kt * 128:(kt + 1) * 128,
                                     p * 256:(p + 1) * 256])
                    return t

                wz = px.tile([1, 640], BF16)
                nc.vector.memset(wz, 0.5)

                def pe_warm(n):
                    # keep the PE busy (and its p-state hot) while x loads
                    for _ in range(n):
                        st = ps.tile([128, 3, 512], F32, tag="s", bufs=2)
                        nc.tensor.matmul(
                            out=st[:, 0, :],
                            lhsT=wz[0:1, 0:128], rhs=wz[0:1, 0:512],
                            start=True, stop=True)

                # ---- warm-up: pair-0 qk (k first, then q by halves so the
                # first scores start as early as possible); pair-0's v columns
                # fill the PE while ACT chews head-0 exps.
                # interleave pair-0 qk chunks with the first score triples:
                # triple g only needs k for tokens 384g..384g+384, so the
                # first exp fires ~10us in instead of ~20
                pe_warm(12)
                qk_proj_part(0, wqk0_t, [(1, 0), (0, 0), (0, 1)])
                scores_exp(0, 0, jtgs=(0,))
                qk_proj_part(0, wqk0_t, [(1, 1)])
                scores_exp(0, 0, jtgs=(1,))
                qk_proj_part(0, wqk0_t, [(1, 2)])
                scores_exp(0, 0, jtgs=(2, 3))
                qk_proj_part(0, wqk0_t, [(1, 3)])
                scores_exp(0, 0, jtgs=(4, 5))
                qk_proj_part(0, wqk0_t, [(0, 2), (0, 3)])
                wqk_t = wqk_load(1)
                scores_exp(0, 1)
                v_proj(0, range(16))

                # ---- steady state half-steps: pv of step k-2 (deps ready),
                # scores of step k (keeps ACT fed), then filler PE work packed
                # as late as its deadline allows (late heads have no fillers
                # left, so deadline-packing minimizes end-game PE idle).
                for k in range(2, 14):
                    h2, qh2 = (k - 2) // 2, (k - 2) % 2
                    h, qh = k // 2, k % 2
                    pv_half(h2, qh2)
                    scores_exp(h, qh)
                    q = k // 4 + 1                 # pair whose qk is due
                    g = k // 4                     # pair whose v cols are due
                    if k % 4 == 2 and q <= 3:
                        qk_proj_part(q, wqk_t, [(1, ic) for ic in range(4)])
                    elif k % 4 == 3 and q <= 3:
                        qk_proj_part(q, wqk_t, [(0, ic) for ic in range(4)])
                    elif k % 4 == 0:
                        if g <= 3:
                            v_proj(g, range(8))
                        if k == 4:
                            wqk_t = wqk_load(2)
                        if k == 8:
                            wqk_t = wqk_load(3)
                    elif k % 4 == 1:
                        if g <= 3:
                            v_proj(g, range(8, 16))
                        if k > 4:                  # pair k//4-1 ctx done
                            transposes(k // 4 - 1, range(16))
                pv_half(6, 0)
                scores_exp(7, 0)
                pv_half(6, 1)

              # px closed: x/wv/wqk SBUF freed for wo/ot
              with tc.tile_pool(name="pl", bufs=1) as pl:
                wo_s = pl.tile([128, 4, E], BF16)
                for ct in range(4):
                    nc.gpsimd.dma_start(
                        out=wo_s[:, ct, :],
                        in_=wo_d[ct * 128:(ct + 1) * 128, :])

                def out_proj(i2):
                    for et in range(8):
                        po = ps.tile([128, 512], F32, tag="w", bufs=2)
                        for ct in range(4):
                            nc.tensor.matmul(
                                out=po,
                                lhsT=wo_s[:, ct, et * 128:(et + 1) * 128],
                                rhs=ctx_s[:, ct, i2 * 512:(i2 + 1) * 512],
                                start=(ct == 0), stop=(ct == 3))
                        ot = pl.tile([128, 512], F32, tag="ot", bufs=4)
                        nc.vector.tensor_scalar_add(ot, po, bout_s[:, et, 0:1])
                        nc.sync.dma_start(
                            out=out_d[et * 128:(et + 1) * 128,
                                      i2 * 512:(i2 + 1) * 512],
                            in_=ot)

                # tail: weave the last scores half with the pv/out-proj
                # chase at sub-half granularity
                scores_exp(7, 1, i2s=(0,))
                pv_part(7, 0, range(4))
                transposes(3, range(4))
                out_proj(0)
                scores_exp(7, 1, i2s=(1,))
                pv_part(7, 0, range(4, 8))
                transposes(3, range(4, 8))
                out_proj(1)
                pv_part(7, 1, range(4))
                transposes(3, range(8, 12))
                out_proj(2)
                pv_part(7, 1, range(4, 8))
                transposes(3, range(12, 16))
                out_proj(3)
    nc.compile()
    return nc


_NC = None


def _get_nc():
    global _NC
    if _NC is None:
        _NC = build_nc()
    return _NC


def make_in_maps(query, Wqkv, bqkv, Wout, bout):
    query = np.asarray(query, dtype=np.float32)
    Wqkv = np.asarray(Wqkv, dtype=np.float32)
    bqkv = np.asarray(bqkv, dtype=np.float32)
    Wout = np.asarray(Wout, dtype=np.float32)
    bout = np.asarray(bout, dtype=np.float32)
    bf = ml_dtypes.bfloat16

    in_maps = []
    for c in range(8):
        b, hh = c // 2, c % 2
        heads = np.arange(hh * HL, hh * HL + HL)
        dims = (heads[:, None] * HD + np.arange(HD)[None, :]).reshape(-1)  # [512]
        q_rows, k_rows, v_rows = dims, E + dims, 2 * E + dims

        xw = np.empty((E, 3584), bf)
        for p in range(4):
            pd = dims[p * 128:(p + 1) * 128]
            xw[:, p * 256:p * 256 + 128] = Wqkv[pd].T.astype(bf)
            xw[:, p * 256 + 128:p * 256 + 256] = Wqkv[E + pd].T.astype(bf)
        xw[:, 1024:3072] = query[b].T.astype(bf)
        xw[:, 3072:3584] = Wqkv[v_rows].T.astype(bf)

        # bqk rows: [q pair0..3 | k pair0..3], each pair-major 128 rows
        bqk = np.concatenate([bqkv[q_rows], bqkv[k_rows]]).reshape(E, 1)

        cons = np.zeros((1, CONS_LEN), bf)
        cons[0, ONES_OFF:ONES_OFF + 128] = 1.0
        cons[0, BV_OFF:BV_OFF + 512] = bqkv[v_rows].astype(bf)
        vpad = np.zeros(520, bf)
        for i in range(8):
            vpad[i * 65 + 64] = 1.0
        cons[0, VPAD_OFF:VPAD_OFF + 520] = vpad

        wo = np.ascontiguousarray(Wout[:, dims].T).astype(bf)   # [512, E]
        bo = (bout if hh == 0 else np.zeros_like(bout)).reshape(E, 1)

        in_maps.append({
            "xw": xw, "bqk": np.ascontiguousarray(bqk),
            "cons": cons, "wo": wo, "bout": np.ascontiguousarray(bo),
        })
    return in_maps


def gather(results):
    out = np.empty((B, S, E), np.float32)
    for b in range(B):
        acc = results[2 * b]["outT"] + results[2 * b + 1]["outT"]   # [E, S]
        out[b] = acc.T
    return out


def kernel(query, key, value, Wqkv, bqkv, Wout, bout):
    # key/value are unused by the reference module (qkv all from query)
    nc = _get_nc()
    in_maps = make_in_maps(query, Wqkv, bqkv, Wout, bout)
    res = run_bass_kernel_spmd(nc, in_maps, list(range(8)))
    return gather(res.results)


# revision 42
# speedup vs baseline: 1.3512x; 1.0074x over previous
"""Multi-head attention (B=4, S=2048, E=1024, H=16) on 8 trn2 NeuronCores.

Sharding: data-parallel over B (4) x tensor-parallel over H (2 halves of 8
heads). Core c handles batch c//2, head-half c%2. Column-parallel qkv_proj,
row-parallel out_proj; the all-reduce of the two partial outputs per batch is
done on the host during unshard (a sum of two arrays), as is the final
transpose (the device emits out^T to keep DMA writes contiguous).

Device kernel v2 (bf16 matmuls, fp32 psum), 300us vs the 403us fp32r
baseline (PE ~277us busy at 92%, ACT exp ~255us). Key structure:
  - scores^T per key-tile-triple in [128 keys, 3 jt, 512 q] psum tiles; one
    1536-wide ACT exp per tile (amortizes ACT access overhead) -> e bf16
    with the 1/sqrt(E) scale folded in.
  - PV runs TRANSPOSED: stationary = e-tile [128 keys, 128 queries], moving
    = [v_h | 1] bf16 [128, 65], psum accumulates [128 q, 64 ctx | softmax
    denominator] - halves PV cycles vs the [65, q] form and makes the
    normalizer a per-partition scalar: reciprocal + tensor_scalar_mul on
    DVE, no DRAM-bounce partition broadcast.
  - ctx [q, d] -> [d, q] via the DMA xbar (dma_start_transpose), zero PE.
  - software-pipelined emission at half-head granularity: qk-proj (JIT per
    pair) and per-pair v-proj columns are deadline-packed as fillers so the
    PE never starves while ACT paces the exps; memset-fed dummy matmuls
    keep the PE p-state hot during the initial x load (split across the
    sync+gpsimd DMA queues); pair-0 qk chunks interleave with the first
    score triples (triple g only needs k-tokens 384g..384g+384) so the
    first exp fires ~12us in; the last head runs 512-query-fine so PV /
    xbar-transpose / out-proj chase the final exps at sub-half granularity.
    All biases ride DVE evictions (qk/out: per-partition tensor_scalar_add;
    v: broadcast-DMA'd row + tensor_add), so no bias matmuls.
"""
import sys

import numpy as np

sys.path.insert(0, "/opt/trn_rl_repo")

import ml_dtypes

import concourse.bacc as bacc
import concourse.mybir as mybir
import concourse.tile as tile
from concourse.bass_utils import run_bass_kernel_spmd

F32 = mybir.dt.float32
BF16 = mybir.dt.bfloat16
EXP = mybir.ActivationFunctionType.Exp

B, S, E, H, HD = 4, 2048, 1024, 16, 64
HL = 8            # heads per core
SCALE = float(1.0 / np.sqrt(np.float32(E)))

# cons layout (bf16 row): [0:128] ones, [128:640] bv, [640:1160] v1 pad row
ONES_OFF, BV_OFF, VPAD_OFF, CONS_LEN = 0, 128, 640, 1160


def build_nc():
    nc = bacc.Bacc("TRN2", target_bir_lowering=False, debug=False, num_devices=8)
    # xw cols: [0:1024] per-pair [Wq_p^T | Wk_p^T] (4 x 256), [1024:3072] x^T,
    # [3072:3584] Wv_loc^T -- all bf16
    xw_d = nc.declare_dram_parameter("xw", [E, 3584], BF16, isOutput=False)
    bqk_d = nc.declare_dram_parameter("bqk", [E, 1], F32, isOutput=False)
    cons_d = nc.declare_dram_parameter("cons", [1, CONS_LEN], BF16, isOutput=False)
    wo_d = nc.declare_dram_parameter("wo", [512, E], BF16, isOutput=False)
    bout_d = nc.declare_dram_parameter("bout", [E, 1], F32, isOutput=False)
    out_d = nc.declare_dram_parameter("outT", [E, S], F32, isOutput=True)

    with tile.TileContext(nc) as tc:
      with tc.tile_pool(name="pp", bufs=1) as pp:
        bqk_s = pp.tile([128, 8, 1], F32)
        bout_s = pp.tile([128, 8, 1], F32)
        nc.gpsimd.dma_start(out=bqk_s, in_=bqk_d[:, :].rearrange("(m p) o -> p m o", p=128))
        # warm the ACT exp table (load is ~1.3us; keep it off the critical path)
        warm = pp.tile([1, 1], F32)
        nc.scalar.activation(out=warm, in_=bqk_s[0:1, 0, 0:1], func=EXP)

        with tc.tile_pool(name="pa", bufs=1) as pa:
            # persistent phase-3 state
            qk_s = pa.tile([128, 2, 2, S], BF16)        # [slot, q|k, tok]
            v1_s = pa.tile([128, 16, 520], BF16)        # per jt: 8 x [v_h(64) | 1]
            e_s = [pa.tile([128, 16, 1024], BF16, name=f"e{i}") for i in range(3)]
            ctq_s = pa.tile([128, 2, 16, 128], BF16)    # [slot, qb, d-pair]
            ctx_s = pa.tile([128, 4, S], BF16)          # ctx^T per pair [d, q]

            with tc.tile_pool(name="ps", bufs=1, space="PSUM") as ps:
              with tc.tile_pool(name="px", bufs=1) as px:
                x_s = px.tile([128, 8, S], BF16)
                wv_s = px.tile([128, 8, 512], BF16)
                wqk0_t = px.tile([128, 8, 256], BF16, tag="wqk", bufs=2)
                for kt in range(8):
                    eng = nc.gpsimd if kt % 2 == 0 else nc.sync
                    eng.dma_start(
                        out=wqk0_t[:, kt, :],
                        in_=xw_d[kt * 128:(kt + 1) * 128, 0:256])
                # ic-major, alternating queues: halves the per-queue DMA
                # issue serialization on the warm-up critical path
                for ic in range(4):
                    for kt in range(8):
                        eng = nc.sync if kt % 2 == 0 else nc.gpsimd
                        eng.dma_start(
                            out=x_s[:, kt, ic * 512:(ic + 1) * 512],
                            in_=xw_d[kt * 128:(kt + 1) * 128,
                                     1024 + ic * 512:1024 + (ic + 1) * 512])
                # after x on the sync queue: keeps the shared DMA engines
                # clear for the critical-path x load (wv/v1 needed ~30us in)
                for kt in range(8):
                    nc.sync.dma_start(
                        out=wv_s[:, kt, :],
                        in_=xw_d[kt * 128:(kt + 1) * 128, 3072:3584])
                for jt in range(16):
                    nc.sync.dma_start(
                        out=v1_s[:, jt, :],
                        in_=cons_d[0:1, VPAD_OFF:VPAD_OFF + 520]
                        .to_broadcast([128, 520]))
                bv_s = px.tile([128, 512], BF16)
                nc.sync.dma_start(
                    out=bv_s,
                    in_=cons_d[0:1, BV_OFF:BV_OFF + 512].to_broadcast([128, 512]))
                nc.sync.dma_start(
                    out=bout_s,
                    in_=bout_d[:, :].rearrange("(m p) o -> p m o", p=128))

                def qk_proj_part(p, wqk_t, chunks):
                    for src, ic in chunks:      # src: 1=k, 0=q; ic: 512 tok
                        pj = ps.tile([128, 512], F32, tag="w", bufs=2)
                        for kt in range(8):
                            nc.tensor.matmul(
                                out=pj,
                                lhsT=wqk_t[:, kt, src * 128:(src + 1) * 128],
                                rhs=x_s[:, kt, ic * 512:(ic + 1) * 512],
                                start=(kt == 0), stop=(kt == 7))
                        nc.vector.tensor_scalar_add(
                            qk_s[:, p % 2, src, ic * 512:(ic + 1) * 512],
                            pj, bqk_s[:, 4 * src + p, 0:1])

                def qk_proj(p, wqk_t):
                    # k first (scores need all of k, only half of q at a time)
                    qk_proj_part(p, wqk_t,
                                 [(1, ic) for ic in range(4)]
                                 + [(0, ic) for ic in range(4)])

                def v_proj(g, jts):
                    # one head-pair's v columns (g = pair index), so the work
                    # spreads across the schedule instead of lumping 34us
                    for jt in jts:
                        pj = ps.tile([128, 512], F32, tag="w", bufs=2)
                        for kt in range(8):
                            nc.tensor.matmul(
                                out=pj[:, 0:128],
                                lhsT=x_s[:, kt, jt * 128:(jt + 1) * 128],
                                rhs=wv_s[:, kt, g * 128:(g + 1) * 128],
                                start=(kt == 0), stop=(kt == 7))
                        nc.vector.tensor_add(
                            v1_s[:, jt, g * 130:(g + 1) * 130]
                            .rearrange("p (h c) -> p h c", c=65)[:, :, 0:64],
                            pj[:, 0:128].rearrange("p (h c) -> p h c", c=64),
                            bv_s[:, g * 128:(g + 1) * 128]
                            .rearrange("p (h c) -> p h c", c=64))

                def scores_exp(h, qh, i2s=(0, 1), jtgs=tuple(range(6))):
                    # s-tiles pack 3 key-tiles x 512 queries: 1536-wide exps
                    # amortize ACT access overhead; i2-granular so PV can
                    # chase by 512-query sub-halves
                    p, hl = h // 2, h % 2
                    part = slice(hl * 64, hl * 64 + 64)
                    eb = e_s[(2 * h + qh) % 3]
                    for i2 in i2s:
                        q0 = qh * 1024 + i2 * 512
                        for jtg in jtgs:           # jt triples, last has 1
                            j0, j1 = 3 * jtg, min(3 * jtg + 3, 16)
                            st = ps.tile([128, 3, 512], F32, tag="s", bufs=2)
                            for j in range(j0, j1):
                                nc.tensor.matmul(
                                    out=st[:, j - j0, :],
                                    lhsT=qk_s[part, p % 2, 1, j * 128:(j + 1) * 128],
                                    rhs=qk_s[part, p % 2, 0, q0:q0 + 512],
                                    start=True, stop=True)
                            nc.scalar.activation(
                                out=eb[:, j0:j1, i2 * 512:(i2 + 1) * 512],
                                in_=st[:, 0:j1 - j0, :],
                                func=EXP, scale=SCALE)

                def pv_part(h, qh, qbs):
                    # two query-blocks share one psum slot; one batched
                    # reciprocal per pair keeps DVE ahead of the PE
                    p, hl = h // 2, h % 2
                    eb = e_s[(2 * h + qh) % 3]
                    qbs = list(qbs)
                    for qq in range(0, len(qbs), 2):
                        pv = ps.tile([128, 2, 256], F32, tag="w", bufs=2)
                        for j2, qb in enumerate(qbs[qq:qq + 2]):
                            for jt in range(16):
                                nc.tensor.matmul(
                                    out=pv[:, j2, 0:65],
                                    lhsT=eb[:, jt, qb * 128:(qb + 1) * 128],
                                    rhs=v1_s[:, jt, h * 65:h * 65 + 65],
                                    start=(jt == 0), stop=(jt == 15))
                        rcp = pa.tile([128, 2], F32, tag="rcp", bufs=4)
                        nc.vector.reciprocal(rcp, pv[:, :, 64])
                        for j2, qb in enumerate(qbs[qq:qq + 2]):
                            nc.vector.tensor_scalar_mul(
                                ctq_s[:, p % 2, qh * 8 + qb, hl * 64:hl * 64 + 64],
                                pv[:, j2, 0:64], rcp[:, j2:j2 + 1])

                def pv_half(h, qh):
                    pv_part(h, qh, range(8))

                def transposes(p, qbs):
                    for qb in qbs:
                        nc.sync.dma_start_transpose(
                            out=ctx_s[:, p, qb * 128:(qb + 1) * 128],
                            in_=ctq_s[:, p % 2, qb, :])

                def wqk_load(p):
                    t = px.tile([128, 8, 256], BF16, tag="wqk", bufs=2)
                    for kt in range(8):
                        nc.sync.dma_start(
                            out=t[:, kt, :],
                            in_=xw_d[kt * 128:(kt + 1) * 128,
                                     p * 256:(p + 1) * 256])
                    return t

                wz = px.tile([1, 640], BF16)
                nc.vector.memset(wz, 0.5)

                def pe_warm(n):
                    # keep the PE busy (and its p-state hot) while x loads
                    for _ in range(n):
                        st = ps.tile([128, 3, 512], F32, tag="s", bufs=2)
                        nc.tensor.matmul(
                            out=st[:, 0, :],
                            lhsT=wz[0:1, 0:128], rhs=wz[0:1, 0:512],
                            start=True, stop=True)

                # ---- warm-up: pair-0 qk (k first, then q by halves so the
                # first scores start as early as possible); pair-0's v columns
                # fill the PE while ACT chews head-0 exps.
                # interleave pair-0 qk chunks with the first score triples:
                # triple g only needs k for tokens 384g..384g+384, so the
                # first exp fires ~10us in instead of ~20
                pe_warm(12)
                qk_proj_part(0, wqk0_t, [(1, 0), (0, 0), (0, 1)])
                scores_exp(0, 0, jtgs=(0,))
                qk_proj_part(0, wqk0_t, [(1, 1)])
                scores_exp(0, 0, jtgs=(1,))
                qk_proj_part(0, wqk0_t, [(1, 2)])
                scores_exp(0, 0, jtgs=(2, 3))
                qk_proj_part(0, wqk0_t, [(1, 3)])
                scores_exp(0, 0, jtgs=(4, 5))
                qk_proj_part(0, wqk0_t, [(0, 2), (0, 3)])
                wqk_t = wqk_load(1)
                scores_exp(0, 1)
                v_proj(0, range(16))

                # ---- steady state half-steps: pv of step k-2 (deps ready),
                # scores of step k (keeps ACT fed), then filler PE work packed
                # as late as its deadline allows (late heads have no fillers
                # left, so deadline-packing minimizes end-game PE idle).
                for k in range(2, 14):
                    h2, qh2 = (k - 2) // 2, (k - 2) % 2
                    h, qh = k // 2, k % 2
                    pv_half(h2, qh2)
                    scores_exp(h, qh)
                    q = k // 4 + 1                 # pair whose qk is due
                    g = k // 4                     # pair whose v cols are due
                    if k % 4 == 2 and q <= 3:
                        qk_proj_part(q, wqk_t, [(1, ic) for ic in range(4)])
                    elif k % 4 == 3 and q <= 3:
                        qk_proj_part(q, wqk_t, [(0, ic) for ic in range(4)])
                    elif k % 4 == 0:
                        if g <= 3:
                            v_proj(g, range(8))
                        if k == 4:
                            wqk_t = wqk_load(2)
                        if k == 8:
                            wqk_t = wqk_load(3)
                    elif k % 4 == 1:
                        if g <= 3:
                            v_proj(g, range(8, 16))
                        if k > 4:                  # pair k//4-1 ctx done
                            transposes(k // 4 - 1, range(16))
                pv_half(6, 0)
                scores_exp(7, 0)
                pv_half(6, 1)

              # px closed: x/wv/wqk SBUF freed for wo/ot
              with tc.tile_pool(name="pl", bufs=1) as pl:
                wo_s = pl.tile([128, 4, E], BF16)
                for ct in range(4):
                    nc.gpsimd.dma_start(
                        out=wo_s[:, ct, :],
                        in_=wo_d[ct * 128:(ct + 1) * 128, :])

                def out_proj(i2):
                    for et in range(8):
                        po = ps.tile([128, 512], F32, tag="w", bufs=2)
                        for ct in range(4):
                            nc.tensor.matmul(
                                out=po,
                                lhsT=wo_s[:, ct, et * 128:(et + 1) * 128],
                                rhs=ctx_s[:, ct, i2 * 512:(i2 + 1) * 512],
                                start=(ct == 0), stop=(ct == 3))
                        ot = pl.tile([128, 512], F32, tag="ot", bufs=4)
                        nc.vector.tensor_scalar_add(ot, po, bout_s[:, et, 0:1])
                        nc.sync.dma_start(
                            out=out_d[et * 128:(et + 1) * 128,
                                      i2 * 512:(i2 + 1) * 512],
                            in_=ot)

                # tail: weave the last scores half with the pv/out-proj
                # chase at sub-half granularity
                scores_exp(7, 1, i2s=(0,))
                pv_part(7, 0, range(4))
                transposes(3, range(4))
                out_proj(0)
                scores_exp(7, 1, i2s=(1,))
                pv_part(7, 0, range(4, 8))
                transposes(3, range(4, 8))
                out_proj(1)
                pv_part(7, 1, range(4))
                transposes(3, range(8, 12))
                out_proj(2)
                pv_part(7, 1, range(4, 8))
                transposes(3, range(12, 16))
                out_proj(3)
    nc.compile()
    return nc


_NC = None


def _get_nc():
    global _NC
    if _NC is None:
        _NC = build_nc()
    return _NC


def make_in_maps(query, Wqkv, bqkv, Wout, bout):
    query = np.asarray(query, dtype=np.float32)
    Wqkv = np.asarray(Wqkv, dtype=np.float32)
    bqkv = np.asarray(bqkv, dtype=np.float32)
    Wout = np.asarray(Wout, dtype=np.float32)
    bout = np.asarray(bout, dtype=np.float32)
    bf = ml_dtypes.bfloat16

    in_maps = []
    for c in range(8):
        b, hh = c // 2, c % 2
        heads = np.arange(hh * HL, hh * HL + HL)
        dims = (heads[:, None] * HD + np.arange(HD)[None, :]).reshape(-1)  # [512]
        q_rows, k_rows, v_rows = dims, E + dims, 2 * E + dims

        xw = np.empty((E, 3584), bf)
        for p in range(4):
            pd = dims[p * 128:(p + 1) * 128]
            xw[:, p * 256:p * 256 + 128] = Wqkv[pd].T.astype(bf)
            xw[:, p * 256 + 128:p * 256 + 256] = Wqkv[E + pd].T.astype(bf)
        xw[:, 1024:3072] = query[b].T.astype(bf)
        xw[:, 3072:3584] = Wqkv[v_rows].T.astype(bf)

        # bqk rows: [q pair0..3 | k pair0..3], each pair-major 128 rows
        bqk = np.concatenate([bqkv[q_rows], bqkv[k_rows]]).reshape(E, 1)

        cons = np.zeros((1, CONS_LEN), bf)
        cons[0, ONES_OFF:ONES_OFF + 128] = 1.0
        cons[0, BV_OFF:BV_OFF + 512] = bqkv[v_rows].astype(bf)
        vpad = np.zeros(520, bf)
        for i in range(8):
            vpad[i * 65 + 64] = 1.0
        cons[0, VPAD_OFF:VPAD_OFF + 520] = vpad

        wo = np.ascontiguousarray(Wout[:, dims].T).astype(bf)   # [512, E]
        bo = (bout if hh == 0 else np.zeros_like(bout)).reshape(E, 1)

        in_maps.append({
            "xw": xw, "bqk": np.ascontiguousarray(bqk),
            "cons": cons, "wo": wo, "bout": np.ascontiguousarray(bo),
        })
    return in_maps


def gather(results):
    out = np.empty((B, S, E), np.float32)
    for b in range(B):
        acc = results[2 * b]["outT"] + results[2 * b + 1]["outT"]   # [E, S]
        out[b] = acc.T
    return out


def kernel(query, key, value, Wqkv, bqkv, Wout, bout):
    # key/value are unused by the reference module (qkv all from query)
    nc = _get_nc()
    in_maps = make_in_maps(query, Wqkv, bqkv, Wout, bout)
    res = run_bass_kernel_spmd(nc, in_maps, list(range(8)))
    return gather(res.results)
